# revision 4
# baseline (speedup 1.0000x reference)
"""Trainium2 Bass kernel for the isotropic-gaussian differentiable renderer.

Math: for pixel p=(x,y) and gaussian g:
    w[g,p] = op_g * exp(-0.5*((x-ax_g)^2+(y-ay_g)^2)/var_g)
    img[p,c] = (sum_g w[g,p]*col_gc) / (sum_g w[g,p] + n_chunks*EPS)

The isotropic RBF is separable: w = op * exp(sx) * exp(sy) with
sx = s*(x-ax)^2, sy = s*(y-ay)^2, s = -0.5/var.  That turns the
268M-element exp into 2*N*128 exps plus matmuls:

  per 128-gaussian chunk:
    PE (fp32): arg[g, 0:128]=sx(g,x), arg[g,128:256]=sy(g,y) via a K=6
               matmul against fixed rows [u^2,u,1|v^2,v,1] (centered coords;
               fp32 needed: the expansion cancels catastrophically)
    ACT      : expxy = exp(arg)  (PSUM->SBUF, batched over chunks)
    DVE      : A[g, c*128+y] = opc[g,c]*expy[g,y]   (4 tensor_scalar ops)
    PE       : acc[x, c*128+y] += expx^T @ A        (accumulated in PSUM)

Sharding: gaussians split 2048/core across 8 cores; every core accumulates
the full 128x128 image; host sums the 8 partials, divides num/den and
reshapes to the reference's [4,3,64,64] tile layout.
"""
import numpy as np

import concourse.bacc as bacc
import concourse.tile as tile
from concourse import mybir
from concourse.bass_utils import run_bass_kernel_spmd

# Problem constants (hardcoded per harness contract)
N_GAUSS = 16384
H = 128
W = 128
FX = 128.0
FY = 128.0
CX = 64.0
CY = 64.0
EPS = 1e-8
N_CORES = 8
G_PER_CORE = N_GAUSS // N_CORES      # 2048
CHUNK = 128                          # gaussians per matmul chunk
N_CHUNKS = G_PER_CORE // CHUNK       # 16
ARG_W = 256                          # per-chunk arg width: 128 x | 128 y
GROUP = 4                            # chunks per exp batch
N_GROUPS = N_CHUNKS // GROUP         # 4
OUT_W = 512                          # (c,y) free width of the accumulator

F32 = mybir.dt.float32
F32R = mybir.dt.float32r
MM_DT = F32R                         # matmul dtype (f32r: 1 cyc/row at N>=256)
KARG = 12                            # arg-matmul contraction: 6 coef rows x hi/lo


def build_program():
    """One SPMD Bass program; every core runs it on its gaussian slice."""
    nc = bacc.Bacc("TRN2", target_bir_lowering=False, debug=False,
                   num_devices=N_CORES)
    # [12, 2048]: K-dim rows (hi/lo split, f32r-representable), cols gaussians.
    coef = nc.dram_tensor("coef", [KARG, G_PER_CORE], F32, kind="ExternalInput")
    # [12, 256]: fixed moving rows [u^2,u^2,u,u,1,1|0...] / [0...|v^2,v^2,v,v,1,1]
    rhsxy = nc.dram_tensor("rhsxy", [KARG, ARG_W], F32, kind="ExternalInput")
    # [128, 64]: opc[p, chunk*4+c] = (op*[r,g,b,1])[chunk*128+p, c]
    opc = nc.dram_tensor("opc", [128, N_CHUNKS * 4], F32, kind="ExternalInput")
    # partial accumulator: [x, c*128+y]
    out = nc.dram_tensor("out", [128, OUT_W], F32, kind="ExternalOutput")

    with tile.TileContext(nc) as tc:
        with tc.tile_pool(name="ins", bufs=1) as ins_pool, \
             tc.tile_pool(name="expp", bufs=1) as exp_pool, \
             tc.tile_pool(name="apool", bufs=3) as a_pool, \
             tc.tile_pool(name="args", bufs=2, space="PSUM") as arg_pool, \
             tc.tile_pool(name="acc", bufs=1, space="PSUM") as acc_pool, \
             tc.tile_pool(name="outp", bufs=1) as out_pool:

            coef_t = ins_pool.tile([KARG, G_PER_CORE], F32)
            rhs_t = ins_pool.tile([KARG, ARG_W], F32)
            opc_t = ins_pool.tile([128, N_CHUNKS * 4], F32)
            # parallel triggers: each input on its own engine's queue
            nc.sync.dma_start(out=coef_t, in_=coef[:, :])
            nc.scalar.dma_start(out=rhs_t, in_=rhsxy[:, :])
            nc.gpsimd.dma_start(out=opc_t, in_=opc[:, :])

            # f32r operands must be produced by a rounding engine op; host
            # values are pre-rounded to f32r grid so these copies are exact.
            coef_r = ins_pool.tile([KARG, G_PER_CORE], F32R)
            rhs_r = ins_pool.tile([KARG, ARG_W], F32R)
            nc.vector.tensor_copy(coef_r, coef_t)
            nc.vector.tensor_copy(rhs_r, rhs_t)

            # exp(arg) results for all chunks: [g_part, chunk*256 + (x|y)]
            expxy = exp_pool.tile([128, N_CHUNKS * ARG_W], MM_DT)
            acc = acc_pool.tile([128, OUT_W], F32)

            for grp in range(N_GROUPS):
                args = arg_pool.tile([128, GROUP * ARG_W], F32, tag="args")
                for k in range(GROUP):
                    chunk = grp * GROUP + k
                    nc.tensor.matmul(
                        args[:, k * ARG_W:(k + 1) * ARG_W],
                        coef_r[:, chunk * CHUNK:(chunk + 1) * CHUNK],
                        rhs_r[:, :],
                        start=True, stop=True,
                    )
                nc.scalar.activation(
                    out=expxy[:, grp * GROUP * ARG_W:(grp + 1) * GROUP * ARG_W],
                    in_=args[:, :],
                    func=mybir.ActivationFunctionType.Exp,
                )

            for chunk in range(N_CHUNKS):
                ex0 = chunk * ARG_W
                a_t = a_pool.tile([128, OUT_W], MM_DT, tag="a")
                # A[g, c*128+y] = opc[g,c] * expy[g,y]; split across DVE/GPSIMD
                for c, eng in ((0, nc.vector), (1, nc.vector),
                               (2, nc.gpsimd), (3, nc.gpsimd)):
                    eng.tensor_scalar_mul(
                        out=a_t[:, c * 128:(c + 1) * 128],
                        in0=expxy[:, ex0 + 128:ex0 + 256],
                        scalar1=opc_t[:, chunk * 4 + c:chunk * 4 + c + 1],
                    )
                nc.tensor.matmul(
                    acc[:, :],
                    expxy[:, ex0:ex0 + 128],
                    a_t[:, :],
                    start=(chunk == 0), stop=(chunk == N_CHUNKS - 1),
                )

            out_t = out_pool.tile([128, OUT_W], F32)
            nc.vector.tensor_copy(out_t, acc)
            nc.sync.dma_start(out=out[:, :], in_=out_t)

    nc.compile()
    return nc


_PROGRAM = None


def _get_program():
    global _PROGRAM
    if _PROGRAM is None:
        _PROGRAM = build_program()
    return _PROGRAM


def _quat2mat(q):
    q = q / np.linalg.norm(q)
    w, x, y, z = q
    return np.array([
        [1 - 2 * (y * y + z * z), 2 * (x * y - z * w), 2 * (x * z + y * w)],
        [2 * (x * y + z * w), 1 - 2 * (x * x + z * z), 2 * (y * z - x * w)],
        [2 * (x * z - y * w), 2 * (y * z + x * w), 1 - 2 * (x * x + y * y)],
    ])


def kernel(positions, colors, opacities, scales, qvec, tvec, tile_hw,
           chunk_gauss, _trace=False):
    positions = np.asarray(positions, dtype=np.float32)
    colors = np.asarray(colors, dtype=np.float32)
    opacities = np.asarray(opacities, dtype=np.float32)
    scales = np.asarray(scales, dtype=np.float32)
    qvec = np.asarray(qvec, dtype=np.float32)
    tvec = np.asarray(tvec, dtype=np.float32)
    tile_hw = int(tile_hw)
    chunk_gauss = int(chunk_gauss)
    n = positions.shape[0]
    assert n == N_GAUSS, f"expected {N_GAUSS} gaussians, got {n}"

    # ---- O(N) per-gaussian prep in float64 (rounds to the same f32 values
    # the reference computes, to well within the exp's own error budget) ----
    R = _quat2mat(qvec.astype(np.float64))
    cam = positions.astype(np.float64) @ R.T + tvec.astype(np.float64)
    ax = cam[:, 0] / cam[:, 2] * FX + CX          # [N] screen x center
    ay = cam[:, 1] / cam[:, 2] * FY + CY          # [N] screen y center
    var = scales[:, 0].astype(np.float64) ** 2
    s = -0.5 / var                                # [N] negative inv 2*var

    # centered coords keep the quadratic-expansion terms small (|u|<=64)
    dx = ax - CX
    dy = ay - CY

    def f32r_round(x):
        """Round to the f32r grid (low 12 mantissa bits of fp32 cleared)."""
        v32 = np.asarray(x, dtype=np.float32).view(np.uint32)
        return ((v32 + 0x800) & np.uint32(0xFFFFF000)).view(np.float32)

    def hilo(x):
        """Split x into f32r-representable hi+lo with hi+lo ~= x to ~2^-24."""
        hi = f32r_round(x).astype(np.float64)
        lo = f32r_round(np.asarray(x, dtype=np.float64) - hi)
        return hi.astype(np.float32), lo

    # rows of the K=12 stationary operand, per gaussian (hi/lo pairs):
    #   arg_x = s*u^2 + (-2 s dx)*u + s*dx^2     (u = x - 64)
    #   arg_y = s*v^2 + (-2 s dy)*v + s*dy^2     (v = y - 64)
    # u^2 <= 4096 is exactly representable in f32r (12-bit significand), so
    # hi-row products are exact in the PE and lo-rows mop up the residue:
    # the f32r matmul then matches fp32 to ~1e-6 despite the cancellation.
    rows6 = [s, -2.0 * s * dx, s * dx * dx,
             s, -2.0 * s * dy, s * dy * dy]
    coef_rows = []
    for r in rows6:
        hi, lo = hilo(r)
        coef_rows.append(hi)
        coef_rows.append(lo)
    coef_full = np.stack(coef_rows).astype(np.float32)   # [12, N]

    u = np.arange(W, dtype=np.float64) - CX
    v = np.arange(H, dtype=np.float64) - CY
    zeros = np.zeros(128)
    ones = np.ones(128)
    rhs_rows = []
    for base in (u * u, u, ones):
        row = np.concatenate([base, zeros])
        rhs_rows.append(row)
        rhs_rows.append(row)          # hi and lo coef rows share the base
    for base in (v * v, v, ones):
        row = np.concatenate([zeros, base])
        rhs_rows.append(row)
        rhs_rows.append(row)
    rhsxy = np.stack(rhs_rows).astype(np.float32)        # [12, 256]
    assert np.array_equal(f32r_round(rhsxy), rhsxy), "rhs not on f32r grid"

    op = opacities[:, 0].astype(np.float64)
    opc_full = np.concatenate(
        [colors.astype(np.float64) * op[:, None], op[:, None]], axis=1
    ).astype(np.float32)                          # [N, 4] = op*[r,g,b,1]

    # ---- shard gaussians across the 8 cores ----
    in_maps = []
    for core in range(N_CORES):
        g0 = core * G_PER_CORE
        g1 = g0 + G_PER_CORE
        opc_c = opc_full[g0:g1].reshape(N_CHUNKS, CHUNK, 4)
        opc_c = np.ascontiguousarray(
            opc_c.transpose(1, 0, 2).reshape(CHUNK, N_CHUNKS * 4))
        in_maps.append({
            "coef": np.ascontiguousarray(coef_full[:, g0:g1]),
            "rhsxy": rhsxy,
            "opc": opc_c,
        })

    nc = _get_program()
    res = run_bass_kernel_spmd(nc, in_maps, list(range(N_CORES)),
                               trace=_trace)

    # ---- host reduction: sum per-core partials, divide, reshape ----
    acc = np.zeros((128, 4, 128), dtype=np.float64)   # [x, c, y]
    for core in range(N_CORES):
        acc += res.results[core]["out"].reshape(128, 4, 128)

    num = acc[:, 0:3, :]                          # [x, c, y]
    n_chunks_ref = n // chunk_gauss
    den = acc[:, 3, :] + n_chunks_ref * EPS       # [x, y]
    img = num / den[:, None, :]                   # [x, c, y]
    img = img.transpose(2, 0, 1).reshape(H * W, 3)  # [p=(y,x), c]

    step = tile_hw * tile_hw
    t = (H * W) // step
    out = img.reshape(t, step, 3).transpose(0, 2, 1).reshape(
        t, 3, tile_hw, tile_hw)
    result = out.astype(np.float32)
    if _trace:
        return result, res
    return result


# revision 14
# speedup vs baseline: 1.3594x; 1.3594x over previous
"""Trainium2 Bass kernel for the isotropic-gaussian differentiable renderer.

Math: for pixel p=(x,y) and gaussian g:
    w[g,p] = op_g * exp(-0.5*((x-ax_g)^2+(y-ay_g)^2)/var_g)
    img[p,c] = (sum_g w[g,p]*col_gc) / (sum_g w[g,p] + n_chunks*EPS)

The isotropic RBF is separable: w = op * exp(sx) * exp(sy) with
sx = s*(x-ax)^2, sy = s*(y-ay)^2, s = -0.5/var.  That turns the
268M-element exp into 2*N*128 exps plus matmuls:

  per 128-gaussian chunk:
    PE (fp32): arg[g, 0:128]=sx(g,x), arg[g,128:256]=sy(g,y) via a K=6
               matmul against fixed rows [u^2,u,1|v^2,v,1] (centered coords;
               fp32 needed: the expansion cancels catastrophically)
    ACT      : expxy = exp(arg)  (PSUM->SBUF, batched over chunks)
    DVE      : A[g, c*128+y] = opc[g,c]*expy[g,y]   (4 tensor_scalar ops)
    PE       : acc[x, c*128+y] += expx^T @ A        (accumulated in PSUM)

Sharding: gaussians split 2048/core across 8 cores; every core accumulates
the full 128x128 image; host sums the 8 partials, divides num/den and
reshapes to the reference's [4,3,64,64] tile layout.
"""
import numpy as np

import concourse.bacc as bacc
import concourse.tile as tile
from concourse import mybir
from concourse.bass_utils import run_bass_kernel_spmd

# Problem constants (hardcoded per harness contract)
N_GAUSS = 16384
H = 128
W = 128
FX = 128.0
FY = 128.0
CX = 64.0
CY = 64.0
EPS = 1e-8
N_CORES = 8
G_PER_CORE = N_GAUSS // N_CORES      # 2048
CHUNK = 128                          # gaussians per matmul chunk
N_CHUNKS = G_PER_CORE // CHUNK       # 16
ARG_W = 256                          # per-chunk arg width: 128 x | 128 y
GROUP = 4                            # chunks per exp batch
N_GROUPS = N_CHUNKS // GROUP         # 4
OUT_W = 512                          # (c,y) free width of the accumulator

F32 = mybir.dt.float32
MM_DT = F32                          # main-accumulation matmul dtype
KARG = 6                             # arg-matmul contraction rows
PACK = 4                             # arg matmuls packed per PE pass (row groups)
USE_PACK = False                     # fp32 + tile_position hangs TRN2; keep off


def build_program():
    """One SPMD Bass program; every core runs it on its gaussian slice."""
    nc = bacc.Bacc("TRN2", target_bir_lowering=False, debug=False,
                   num_devices=N_CORES)
    # packed: [128, 4*128]: coefpack[32k+r, grp*128+j] = coef row r of chunk
    # (grp*PACK+k), gaussian j — four chunks stacked at partition 0/32/64/96
    # so four K=6 arg matmuls run concurrently in separate PE row groups.
    # unpacked: [6, 2048] flat, one chunk per 128 columns.
    coef_shape = [128, N_GROUPS * CHUNK] if USE_PACK else [KARG, G_PER_CORE]
    coef = nc.dram_tensor("coef", coef_shape, F32, kind="ExternalInput")
    # the 6 fixed moving rows [u^2,u,1|0] / [0|v^2,v,1] (replicated at
    # partition bands 0/32/64/96 when packed).
    rhs_shape = [128, ARG_W] if USE_PACK else [KARG, ARG_W]
    rhsxy = nc.dram_tensor("rhsxy", rhs_shape, F32, kind="ExternalInput")
    # [128, 64]: opc[p, chunk*4+c] = (op*[r,g,b,1])[chunk*128+p, c]
    opc = nc.dram_tensor("opc", [128, N_CHUNKS * 4], F32, kind="ExternalInput")
    # partial accumulator: [x, c*128+y]
    out = nc.dram_tensor("out", [128, OUT_W], F32, kind="ExternalOutput")

    with tile.TileContext(nc) as tc:
        with tc.tile_pool(name="ins", bufs=1) as ins_pool, \
             tc.tile_pool(name="expp", bufs=1) as exp_pool, \
             tc.tile_pool(name="apool", bufs=3) as a_pool, \
             tc.tile_pool(name="args", bufs=2, space="PSUM") as arg_pool, \
             tc.tile_pool(name="acc", bufs=1, space="PSUM") as acc_pool, \
             tc.tile_pool(name="outp", bufs=1) as out_pool:

            coef_t = ins_pool.tile(coef_shape, F32)
            rhs_t = ins_pool.tile(rhs_shape, F32)
            opc_t = ins_pool.tile([128, N_CHUNKS * 4], F32)
            # parallel triggers: each input on its own engine's queue
            nc.sync.dma_start(out=coef_t, in_=coef[:, :])
            nc.scalar.dma_start(out=rhs_t, in_=rhsxy[:, :])
            nc.gpsimd.dma_start(out=opc_t, in_=opc[:, :])

            # exp(arg) results for all chunks: [g_part, chunk*256 + (x|y)]
            expxy = exp_pool.tile([128, N_CHUNKS * ARG_W], MM_DT)
            acc = acc_pool.tile([128, OUT_W], F32)

            for grp in range(N_GROUPS):
                args = arg_pool.tile([128, GROUP * ARG_W], F32, tag="args")
                for k in range(PACK):
                    chunk = grp * PACK + k
                    if USE_PACK:
                        bp = 32 * k
                        lhsT = coef_t[bp:bp + KARG,
                                      grp * CHUNK:(grp + 1) * CHUNK]
                        rhs = rhs_t[bp:bp + KARG, :]
                        tp = (bp, 0)
                    else:
                        lhsT = coef_t[:, chunk * CHUNK:(chunk + 1) * CHUNK]
                        rhs = rhs_t[:, :]
                        tp = None
                    nc.tensor.matmul(
                        args[:, k * ARG_W:(k + 1) * ARG_W],
                        lhsT, rhs,
                        start=True, stop=True,
                        tile_position=tp,
                    )
                nc.scalar.activation(
                    out=expxy[:, grp * GROUP * ARG_W:(grp + 1) * GROUP * ARG_W],
                    in_=args[:, :],
                    func=mybir.ActivationFunctionType.Exp,
                )

            for chunk in range(N_CHUNKS):
                ex0 = chunk * ARG_W
                a_t = a_pool.tile([128, OUT_W], MM_DT, tag="a")
                # A[g, c*128+y] = opc[g,c] * expy[g,y]
                for c, eng in ((0, nc.vector), (1, nc.vector),
                               (2, nc.vector), (3, nc.gpsimd)):
                    eng.tensor_scalar_mul(
                        out=a_t[:, c * 128:(c + 1) * 128],
                        in0=expxy[:, ex0 + 128:ex0 + 256],
                        scalar1=opc_t[:, chunk * 4 + c:chunk * 4 + c + 1],
                    )
                nc.tensor.matmul(
                    acc[:, :],
                    expxy[:, ex0:ex0 + 128],
                    a_t[:, :],
                    start=(chunk == 0), stop=(chunk == N_CHUNKS - 1),
                )

            out_t = out_pool.tile([128, OUT_W], F32)
            nc.scalar.copy(out=out_t, in_=acc)
            nc.sync.dma_start(out=out[:, :], in_=out_t)

    nc.compile()
    return nc


_PROGRAM = None


def _get_program():
    global _PROGRAM
    if _PROGRAM is None:
        _PROGRAM = build_program()
    return _PROGRAM


def _quat2mat(q):
    q = q / np.linalg.norm(q)
    w, x, y, z = q
    return np.array([
        [1 - 2 * (y * y + z * z), 2 * (x * y - z * w), 2 * (x * z + y * w)],
        [2 * (x * y + z * w), 1 - 2 * (x * x + z * z), 2 * (y * z - x * w)],
        [2 * (x * z - y * w), 2 * (y * z + x * w), 1 - 2 * (x * x + y * y)],
    ])


def kernel(positions, colors, opacities, scales, qvec, tvec, tile_hw,
           chunk_gauss, _trace=False):
    positions = np.asarray(positions, dtype=np.float32)
    colors = np.asarray(colors, dtype=np.float32)
    opacities = np.asarray(opacities, dtype=np.float32)
    scales = np.asarray(scales, dtype=np.float32)
    qvec = np.asarray(qvec, dtype=np.float32)
    tvec = np.asarray(tvec, dtype=np.float32)
    tile_hw = int(tile_hw)
    chunk_gauss = int(chunk_gauss)
    n = positions.shape[0]
    assert n == N_GAUSS, f"expected {N_GAUSS} gaussians, got {n}"

    # ---- O(N) per-gaussian prep in float64 (rounds to the same f32 values
    # the reference computes, to well within the exp's own error budget) ----
    R = _quat2mat(qvec.astype(np.float64))
    cam = positions.astype(np.float64) @ R.T + tvec.astype(np.float64)
    ax = cam[:, 0] / cam[:, 2] * FX + CX          # [N] screen x center
    ay = cam[:, 1] / cam[:, 2] * FY + CY          # [N] screen y center
    var = scales[:, 0].astype(np.float64) ** 2
    s = -0.5 / var                                # [N] negative inv 2*var

    # centered coords keep the quadratic-expansion terms small (|u|<=64)
    dx = ax - CX
    dy = ay - CY

    # rows of the K=6 stationary operand, per gaussian:
    #   arg_x = s*u^2 + (-2 s dx)*u + s*dx^2     (u = x - 64)
    #   arg_y = s*v^2 + (-2 s dy)*v + s*dy^2     (v = y - 64)
    coef_full = np.stack([
        s, -2.0 * s * dx, s * dx * dx,
        s, -2.0 * s * dy, s * dy * dy,
    ]).astype(np.float32)                         # [6, N]

    u = np.arange(W, dtype=np.float64) - CX
    v = np.arange(H, dtype=np.float64) - CY
    zeros = np.zeros(128)
    ones = np.ones(128)
    rhs6 = np.stack([
        np.concatenate([u * u, zeros]),
        np.concatenate([u, zeros]),
        np.concatenate([ones, zeros]),
        np.concatenate([zeros, v * v]),
        np.concatenate([zeros, v]),
        np.concatenate([zeros, ones]),
    ]).astype(np.float32)                         # [6, 256]
    if USE_PACK:
        # replicate at partition bands 0/32/64/96 for the row-group packing
        rhsxy = np.zeros((128, ARG_W), dtype=np.float32)
        for k in range(PACK):
            rhsxy[32 * k:32 * k + KARG] = rhs6
    else:
        rhsxy = rhs6

    op = opacities[:, 0].astype(np.float64)
    opc_full = np.concatenate(
        [colors.astype(np.float64) * op[:, None], op[:, None]], axis=1
    ).astype(np.float32)                          # [N, 4] = op*[r,g,b,1]

    # ---- shard gaussians across the 8 cores ----
    in_maps = []
    for core in range(N_CORES):
        g0 = core * G_PER_CORE
        g1 = g0 + G_PER_CORE
        opc_c = opc_full[g0:g1].reshape(N_CHUNKS, CHUNK, 4)
        opc_c = np.ascontiguousarray(
            opc_c.transpose(1, 0, 2).reshape(CHUNK, N_CHUNKS * 4))
        if USE_PACK:
            # coefpack[32k+r, grp*128+j] = coef row r of chunk grp*PACK+k
            cc = coef_full[:, g0:g1].reshape(6, N_GROUPS, PACK, CHUNK)
            coefpack = np.zeros((128, N_GROUPS * CHUNK), dtype=np.float32)
            for k in range(PACK):
                coefpack[32 * k:32 * k + KARG] = (
                    cc[:, :, k, :].reshape(6, N_GROUPS * CHUNK))
        else:
            coefpack = np.ascontiguousarray(coef_full[:, g0:g1])
        in_maps.append({
            "coef": coefpack,
            "rhsxy": rhsxy,
            "opc": opc_c,
        })

    nc = _get_program()
    res = run_bass_kernel_spmd(nc, in_maps, list(range(N_CORES)),
                               trace=_trace)

    # ---- host reduction: sum per-core partials, divide, reshape ----
    acc = np.zeros((128, 4, 128), dtype=np.float64)   # [x, c, y]
    for core in range(N_CORES):
        acc += res.results[core]["out"].reshape(128, 4, 128)

    num = acc[:, 0:3, :]                          # [x, c, y]
    n_chunks_ref = n // chunk_gauss
    den = acc[:, 3, :] + n_chunks_ref * EPS       # [x, y]
    img = num / den[:, None, :]                   # [x, c, y]
    img = img.transpose(2, 0, 1).reshape(H * W, 3)  # [p=(y,x), c]

    step = tile_hw * tile_hw
    t = (H * W) // step
    out = img.reshape(t, step, 3).transpose(0, 2, 1).reshape(
        t, 3, tile_hw, tile_hw)
    result = out.astype(np.float32)
    if _trace:
        return result, res
    return result


# revision 17
# speedup vs baseline: 2.1539x; 1.5845x over previous
"""Trainium2 Bass kernel for the isotropic-gaussian differentiable renderer.

Math: for pixel p=(x,y) and gaussian g:
    w[g,p] = op_g * exp(-0.5*((x-ax_g)^2+(y-ay_g)^2)/var_g)
    img[p,c] = (sum_g w[g,p]*col_gc) / (sum_g w[g,p] + n_chunks*EPS)

The isotropic RBF is separable: w = op * exp(sx) * exp(sy) with
sx = s*(x-ax)^2, sy = s*(y-ay)^2, s = -0.5/var.  That turns the
268M-element exp into 2*N*128 exps plus matmuls:

  per 128-gaussian chunk:
    PE (fp32): arg[g, 0:128]=sx(g,x), arg[g,128:256]=sy(g,y) via a K=6
               matmul against fixed rows [u^2,u,1|v^2,v,1] (centered coords;
               fp32 needed: the expansion cancels catastrophically)
    ACT      : expxy = exp(arg)  (PSUM->SBUF, batched over chunks)
    DVE      : A[g, c*128+y] = opc[g,c]*expy[g,y]   (4 tensor_scalar ops)
    PE       : acc[x, c*128+y] += expx^T @ A        (accumulated in PSUM)

Sharding: gaussians split 2048/core across 8 cores; every core accumulates
the full 128x128 image; host sums the 8 partials, divides num/den and
reshapes to the reference's [4,3,64,64] tile layout.
"""
import numpy as np

import concourse.bacc as bacc
import concourse.tile as tile
from concourse import mybir
from concourse.bass_utils import run_bass_kernel_spmd

# Problem constants (hardcoded per harness contract)
N_GAUSS = 16384
H = 128
W = 128
FX = 128.0
FY = 128.0
CX = 64.0
CY = 64.0
EPS = 1e-8
N_CORES = 8
G_PER_CORE = N_GAUSS // N_CORES      # 2048
CHUNK = 128                          # gaussians per matmul chunk
N_CHUNKS = G_PER_CORE // CHUNK       # 16
ARG_W = 256                          # per-chunk arg width: 128 x | 128 y
GROUP = 4                            # chunks per exp batch
N_GROUPS = N_CHUNKS // GROUP         # 4
OUT_W = 512                          # (c,y) free width of the accumulator

F32 = mybir.dt.float32
MM_DT = mybir.dt.float16             # main-accumulation matmul dtype.
# fp16 is safe here because of how A is factored: B = op*expy is rounded
# once and BOTH num and den consume the same rounded B (and the same
# rounded expx), so weight-rounding cancels in num/den; only the color
# weights carry an independent 2^-11 rounding, which averages out.
KARG = 6                             # arg-matmul contraction rows
PACK = 4                             # arg matmuls packed per PE pass (row groups)
USE_PACK = False                     # fp32 + tile_position hangs TRN2; keep off


def build_program():
    """One SPMD Bass program; every core runs it on its gaussian slice."""
    nc = bacc.Bacc("TRN2", target_bir_lowering=False, debug=False,
                   num_devices=N_CORES)
    # packed: [128, 4*128]: coefpack[32k+r, grp*128+j] = coef row r of chunk
    # (grp*PACK+k), gaussian j — four chunks stacked at partition 0/32/64/96
    # so four K=6 arg matmuls run concurrently in separate PE row groups.
    # unpacked: [6, 2048] flat, one chunk per 128 columns.
    coef_shape = [128, N_GROUPS * CHUNK] if USE_PACK else [KARG, G_PER_CORE]
    coef = nc.dram_tensor("coef", coef_shape, F32, kind="ExternalInput")
    # the 6 fixed moving rows [u^2,u,1|0] / [0|v^2,v,1] (replicated at
    # partition bands 0/32/64/96 when packed).
    rhs_shape = [128, ARG_W] if USE_PACK else [KARG, ARG_W]
    rhsxy = nc.dram_tensor("rhsxy", rhs_shape, F32, kind="ExternalInput")
    # [128, 64]: opc[p, chunk*4+c] = (op*[r,g,b,1])[chunk*128+p, c]
    opc = nc.dram_tensor("opc", [128, N_CHUNKS * 4], F32, kind="ExternalInput")
    # partial accumulator: [x, c*128+y]
    out = nc.dram_tensor("out", [128, OUT_W], F32, kind="ExternalOutput")

    with tile.TileContext(nc) as tc:
        with tc.tile_pool(name="ins", bufs=1) as ins_pool, \
             tc.tile_pool(name="expp", bufs=1) as exp_pool, \
             tc.tile_pool(name="apool", bufs=3) as a_pool, \
             tc.tile_pool(name="args", bufs=2, space="PSUM") as arg_pool, \
             tc.tile_pool(name="acc", bufs=1, space="PSUM") as acc_pool, \
             tc.tile_pool(name="outp", bufs=1) as out_pool:

            coef_t = ins_pool.tile(coef_shape, F32)
            rhs_t = ins_pool.tile(rhs_shape, F32)
            opc_t = ins_pool.tile([128, N_CHUNKS * 4], F32)
            # parallel triggers: each input on its own engine's queue
            nc.sync.dma_start(out=coef_t, in_=coef[:, :])
            nc.scalar.dma_start(out=rhs_t, in_=rhsxy[:, :])
            nc.gpsimd.dma_start(out=opc_t, in_=opc[:, :])

            # exp(arg) results for all chunks: [g_part, chunk*256 + (x|y)]
            expxy = exp_pool.tile([128, N_CHUNKS * ARG_W], MM_DT)
            acc = acc_pool.tile([128, OUT_W], F32)

            for grp in range(N_GROUPS):
                args = arg_pool.tile([128, GROUP * ARG_W], F32, tag="args")
                for k in range(PACK):
                    chunk = grp * PACK + k
                    if USE_PACK:
                        bp = 32 * k
                        lhsT = coef_t[bp:bp + KARG,
                                      grp * CHUNK:(grp + 1) * CHUNK]
                        rhs = rhs_t[bp:bp + KARG, :]
                        tp = (bp, 0)
                    else:
                        lhsT = coef_t[:, chunk * CHUNK:(chunk + 1) * CHUNK]
                        rhs = rhs_t[:, :]
                        tp = None
                    nc.tensor.matmul(
                        args[:, k * ARG_W:(k + 1) * ARG_W],
                        lhsT, rhs,
                        start=True, stop=True,
                        tile_position=tp,
                    )
                nc.scalar.activation(
                    out=expxy[:, grp * GROUP * ARG_W:(grp + 1) * GROUP * ARG_W],
                    in_=args[:, :],
                    func=mybir.ActivationFunctionType.Exp,
                )

            for chunk in range(N_CHUNKS):
                ex0 = chunk * ARG_W
                a_t = a_pool.tile([128, OUT_W], MM_DT, tag="a")
                # B = op*expy into the den column block, then the color
                # blocks from the ROUNDED B so num/den rounding cancels.
                nc.vector.tensor_scalar_mul(
                    out=a_t[:, 384:512],
                    in0=expxy[:, ex0 + 128:ex0 + 256],
                    scalar1=opc_t[:, chunk * 4 + 3:chunk * 4 + 4],
                )
                for c in range(3):
                    nc.vector.tensor_scalar_mul(
                        out=a_t[:, c * 128:(c + 1) * 128],
                        in0=a_t[:, 384:512],
                        scalar1=opc_t[:, chunk * 4 + c:chunk * 4 + c + 1],
                    )
                nc.tensor.matmul(
                    acc[:, :],
                    expxy[:, ex0:ex0 + 128],
                    a_t[:, :],
                    start=(chunk == 0), stop=(chunk == N_CHUNKS - 1),
                )

            out_t = out_pool.tile([128, OUT_W], F32)
            nc.scalar.copy(out=out_t, in_=acc)
            nc.sync.dma_start(out=out[:, :], in_=out_t)

    nc.compile()
    return nc


_PROGRAM = None


def _get_program():
    global _PROGRAM
    if _PROGRAM is None:
        _PROGRAM = build_program()
    return _PROGRAM


def _quat2mat(q):
    q = q / np.linalg.norm(q)
    w, x, y, z = q
    return np.array([
        [1 - 2 * (y * y + z * z), 2 * (x * y - z * w), 2 * (x * z + y * w)],
        [2 * (x * y + z * w), 1 - 2 * (x * x + z * z), 2 * (y * z - x * w)],
        [2 * (x * z - y * w), 2 * (y * z + x * w), 1 - 2 * (x * x + y * y)],
    ])


def kernel(positions, colors, opacities, scales, qvec, tvec, tile_hw,
           chunk_gauss, _trace=False):
    positions = np.asarray(positions, dtype=np.float32)
    colors = np.asarray(colors, dtype=np.float32)
    opacities = np.asarray(opacities, dtype=np.float32)
    scales = np.asarray(scales, dtype=np.float32)
    qvec = np.asarray(qvec, dtype=np.float32)
    tvec = np.asarray(tvec, dtype=np.float32)
    tile_hw = int(tile_hw)
    chunk_gauss = int(chunk_gauss)
    n = positions.shape[0]
    assert n == N_GAUSS, f"expected {N_GAUSS} gaussians, got {n}"

    # ---- O(N) per-gaussian prep in float64 (rounds to the same f32 values
    # the reference computes, to well within the exp's own error budget) ----
    R = _quat2mat(qvec.astype(np.float64))
    cam = positions.astype(np.float64) @ R.T + tvec.astype(np.float64)
    ax = cam[:, 0] / cam[:, 2] * FX + CX          # [N] screen x center
    ay = cam[:, 1] / cam[:, 2] * FY + CY          # [N] screen y center
    var = scales[:, 0].astype(np.float64) ** 2
    s = -0.5 / var                                # [N] negative inv 2*var

    # centered coords keep the quadratic-expansion terms small (|u|<=64)
    dx = ax - CX
    dy = ay - CY

    # rows of the K=6 stationary operand, per gaussian:
    #   arg_x = s*u^2 + (-2 s dx)*u + s*dx^2     (u = x - 64)
    #   arg_y = s*v^2 + (-2 s dy)*v + s*dy^2     (v = y - 64)
    coef_full = np.stack([
        s, -2.0 * s * dx, s * dx * dx,
        s, -2.0 * s * dy, s * dy * dy,
    ]).astype(np.float32)                         # [6, N]

    u = np.arange(W, dtype=np.float64) - CX
    v = np.arange(H, dtype=np.float64) - CY
    zeros = np.zeros(128)
    ones = np.ones(128)
    rhs6 = np.stack([
        np.concatenate([u * u, zeros]),
        np.concatenate([u, zeros]),
        np.concatenate([ones, zeros]),
        np.concatenate([zeros, v * v]),
        np.concatenate([zeros, v]),
        np.concatenate([zeros, ones]),
    ]).astype(np.float32)                         # [6, 256]
    if USE_PACK:
        # replicate at partition bands 0/32/64/96 for the row-group packing
        rhsxy = np.zeros((128, ARG_W), dtype=np.float32)
        for k in range(PACK):
            rhsxy[32 * k:32 * k + KARG] = rhs6
    else:
        rhsxy = rhs6

    # [N, 4] = [r, g, b, op]: op goes into B = op*expy; colors multiply the
    # already-rounded B (see kernel comment on fp16 rounding cancellation)
    op = opacities[:, 0].astype(np.float64)
    opc_full = np.concatenate(
        [colors.astype(np.float64), op[:, None]], axis=1
    ).astype(np.float32)

    # ---- shard gaussians across the 8 cores ----
    in_maps = []
    for core in range(N_CORES):
        g0 = core * G_PER_CORE
        g1 = g0 + G_PER_CORE
        opc_c = opc_full[g0:g1].reshape(N_CHUNKS, CHUNK, 4)
        opc_c = np.ascontiguousarray(
            opc_c.transpose(1, 0, 2).reshape(CHUNK, N_CHUNKS * 4))
        if USE_PACK:
            # coefpack[32k+r, grp*128+j] = coef row r of chunk grp*PACK+k
            cc = coef_full[:, g0:g1].reshape(6, N_GROUPS, PACK, CHUNK)
            coefpack = np.zeros((128, N_GROUPS * CHUNK), dtype=np.float32)
            for k in range(PACK):
                coefpack[32 * k:32 * k + KARG] = (
                    cc[:, :, k, :].reshape(6, N_GROUPS * CHUNK))
        else:
            coefpack = np.ascontiguousarray(coef_full[:, g0:g1])
        in_maps.append({
            "coef": coefpack,
            "rhsxy": rhsxy,
            "opc": opc_c,
        })

    nc = _get_program()
    res = run_bass_kernel_spmd(nc, in_maps, list(range(N_CORES)),
                               trace=_trace)

    # ---- host reduction: sum per-core partials, divide, reshape ----
    acc = np.zeros((128, 4, 128), dtype=np.float64)   # [x, c, y]
    for core in range(N_CORES):
        acc += res.results[core]["out"].reshape(128, 4, 128)

    num = acc[:, 0:3, :]                          # [x, c, y]
    n_chunks_ref = n // chunk_gauss
    den = acc[:, 3, :] + n_chunks_ref * EPS       # [x, y]
    img = num / den[:, None, :]                   # [x, c, y]
    img = img.transpose(2, 0, 1).reshape(H * W, 3)  # [p=(y,x), c]

    step = tile_hw * tile_hw
    t = (H * W) // step
    out = img.reshape(t, step, 3).transpose(0, 2, 1).reshape(
        t, 3, tile_hw, tile_hw)
    result = out.astype(np.float32)
    if _trace:
        return result, res
    return result


# revision 22
# speedup vs baseline: 2.6390x; 1.2253x over previous
"""Trainium2 Bass kernel for the isotropic-gaussian differentiable renderer.

Math: for pixel p=(x,y) and gaussian g:
    w[g,p] = op_g * exp(-0.5*((x-ax_g)^2+(y-ay_g)^2)/var_g)
    img[p,c] = (sum_g w[g,p]*col_gc) / (sum_g w[g,p] + n_chunks*EPS)

The isotropic RBF is separable: w = op * exp(sx) * exp(sy) with
sx = s*(x-ax)^2, sy = s*(y-ay)^2, s = -0.5/var.  That turns the
268M-element exp into 2*N*128 exps plus matmuls:

  per 128-gaussian chunk:
    PE (fp32): arg[g, 0:128]=sx(g,x), arg[g,128:256]=sy(g,y) via a K=6
               matmul against fixed rows [u^2,u,1|v^2,v,1] (centered coords;
               fp32 needed: the expansion cancels catastrophically)
    ACT      : expxy = exp(arg)  (PSUM->SBUF, batched over chunks)
    DVE      : A[g, c*128+y] = opc[g,c]*expy[g,y]   (4 tensor_scalar ops)
    PE       : acc[x, c*128+y] += expx^T @ A        (accumulated in PSUM)

Sharding: gaussians split 2048/core across 8 cores; every core accumulates
the full 128x128 image; host sums the 8 partials, divides num/den and
reshapes to the reference's [4,3,64,64] tile layout.
"""
import numpy as np

import concourse.bacc as bacc
import concourse.tile as tile
from concourse import mybir
from concourse.bass_utils import run_bass_kernel_spmd

# Problem constants (hardcoded per harness contract)
N_GAUSS = 16384
H = 128
W = 128
FX = 128.0
FY = 128.0
CX = 64.0
CY = 64.0
EPS = 1e-8
N_CORES = 8
G_PER_CORE = N_GAUSS // N_CORES      # 2048
CHUNK = 128                          # gaussians per matmul chunk
N_CHUNKS = G_PER_CORE // CHUNK       # 16
ARG_W = 256                          # per-chunk arg width: 128 x | 128 y
GROUP = 4                            # chunks per exp batch
N_GROUPS = N_CHUNKS // GROUP         # 4
OUT_W = 512                          # (c,y) free width of the accumulator

F32 = mybir.dt.float32
MM_DT = mybir.dt.float16             # main-accumulation matmul dtype.
# fp16 is safe here because of how A is factored: B = op*expy is rounded
# once and BOTH num and den consume the same rounded B (and the same
# rounded expx), so weight-rounding cancels in num/den; only the color
# weights carry an independent 2^-11 rounding, which averages out.
F32R = mybir.dt.float32r
KARG = 12                            # arg-matmul contraction: 6 coef rows x hi/lo
PACK = 4                             # arg matmuls packed per PE pass (row groups)
USE_PACK = False                     # fp32 + tile_position hangs TRN2; keep off


def build_program():
    """One SPMD Bass program; every core runs it on its gaussian slice."""
    nc = bacc.Bacc("TRN2", target_bir_lowering=False, debug=False,
                   num_devices=N_CORES)
    # packed: [128, 4*128]: coefpack[32k+r, grp*128+j] = coef row r of chunk
    # (grp*PACK+k), gaussian j — four chunks stacked at partition 0/32/64/96
    # so four K=6 arg matmuls run concurrently in separate PE row groups.
    # unpacked: [6, 2048] flat, one chunk per 128 columns.
    coef_shape = [128, N_GROUPS * CHUNK] if USE_PACK else [KARG, G_PER_CORE]
    coef = nc.dram_tensor("coef", coef_shape, F32, kind="ExternalInput")
    # the 6 fixed moving rows [u^2,u,1|0] / [0|v^2,v,1] (replicated at
    # partition bands 0/32/64/96 when packed).
    rhs_shape = [128, ARG_W] if USE_PACK else [KARG, ARG_W]
    rhsxy = nc.dram_tensor("rhsxy", rhs_shape, F32, kind="ExternalInput")
    # [128, 64]: opc[p, chunk*4+c] = (op*[r,g,b,1])[chunk*128+p, c]
    opc = nc.dram_tensor("opc", [128, N_CHUNKS * 4], F32, kind="ExternalInput")
    # partial accumulator: [x, c*128+y]
    out = nc.dram_tensor("out", [128, OUT_W], F32, kind="ExternalOutput")

    with tile.TileContext(nc) as tc:
        with tc.tile_pool(name="ins", bufs=1) as ins_pool, \
             tc.tile_pool(name="expp", bufs=1) as exp_pool, \
             tc.tile_pool(name="apool", bufs=3) as a_pool, \
             tc.tile_pool(name="args", bufs=2, space="PSUM") as arg_pool, \
             tc.tile_pool(name="acc", bufs=1, space="PSUM") as acc_pool, \
             tc.tile_pool(name="outp", bufs=1) as out_pool:

            coef_t = ins_pool.tile(coef_shape, F32)
            rhs_t = ins_pool.tile(rhs_shape, F32)
            opc_t = ins_pool.tile([128, N_CHUNKS * 4], F32)
            # parallel triggers: each input on its own engine's queue
            nc.sync.dma_start(out=coef_t, in_=coef[:, :])
            nc.scalar.dma_start(out=rhs_t, in_=rhsxy[:, :])
            nc.gpsimd.dma_start(out=opc_t, in_=opc[:, :])

            # f32r operands must be produced by an on-chip rounding op; the
            # host pre-rounds to the f32r grid so these casts are exact.
            coef_r = ins_pool.tile(coef_shape, F32R)
            rhs_r = ins_pool.tile(rhs_shape, F32R)
            nc.vector.tensor_copy(coef_r, coef_t)
            nc.vector.tensor_copy(rhs_r, rhs_t)

            # exp(arg) results for all chunks: [g_part, chunk*256 + (x|y)]
            expxy = exp_pool.tile([128, N_CHUNKS * ARG_W], MM_DT)
            acc = acc_pool.tile([128, OUT_W], F32)

            for grp in range(N_GROUPS):
                args = arg_pool.tile([128, GROUP * ARG_W], F32, tag="args")
                for k in range(PACK):
                    chunk = grp * PACK + k
                    if USE_PACK:
                        bp = 32 * k
                        lhsT = coef_r[bp:bp + KARG,
                                      grp * CHUNK:(grp + 1) * CHUNK]
                        rhs = rhs_r[bp:bp + KARG, :]
                        tp = (bp, 0)
                    else:
                        lhsT = coef_r[:, chunk * CHUNK:(chunk + 1) * CHUNK]
                        rhs = rhs_r[:, :]
                        tp = None
                    nc.tensor.matmul(
                        args[:, k * ARG_W:(k + 1) * ARG_W],
                        lhsT, rhs,
                        start=True, stop=True,
                        tile_position=tp,
                    )
                nc.scalar.activation(
                    out=expxy[:, grp * GROUP * ARG_W:(grp + 1) * GROUP * ARG_W],
                    in_=args[:, :],
                    func=mybir.ActivationFunctionType.Exp,
                )

            for chunk in range(N_CHUNKS):
                ex0 = chunk * ARG_W
                a_t = a_pool.tile([128, OUT_W], MM_DT, tag="a")
                # B = op*expy into the den column block, then the color
                # blocks from the ROUNDED B so num/den rounding cancels.
                nc.vector.tensor_scalar_mul(
                    out=a_t[:, 384:512],
                    in0=expxy[:, ex0 + 128:ex0 + 256],
                    scalar1=opc_t[:, chunk * 4 + 3:chunk * 4 + 4],
                )
                for c in range(3):
                    nc.vector.tensor_scalar_mul(
                        out=a_t[:, c * 128:(c + 1) * 128],
                        in0=a_t[:, 384:512],
                        scalar1=opc_t[:, chunk * 4 + c:chunk * 4 + c + 1],
                    )
                nc.tensor.matmul(
                    acc[:, :],
                    expxy[:, ex0:ex0 + 128],
                    a_t[:, :],
                    start=(chunk == 0), stop=(chunk == N_CHUNKS - 1),
                )

            out_t = out_pool.tile([128, OUT_W], F32)
            nc.scalar.copy(out=out_t, in_=acc)
            nc.sync.dma_start(out=out[:, :], in_=out_t)

    nc.compile()
    return nc


_PROGRAM = None


def _get_program():
    global _PROGRAM
    if _PROGRAM is None:
        _PROGRAM = build_program()
    return _PROGRAM


def _quat2mat(q):
    q = q / np.linalg.norm(q)
    w, x, y, z = q
    return np.array([
        [1 - 2 * (y * y + z * z), 2 * (x * y - z * w), 2 * (x * z + y * w)],
        [2 * (x * y + z * w), 1 - 2 * (x * x + z * z), 2 * (y * z - x * w)],
        [2 * (x * z - y * w), 2 * (y * z + x * w), 1 - 2 * (x * x + y * y)],
    ])


def kernel(positions, colors, opacities, scales, qvec, tvec, tile_hw,
           chunk_gauss, _trace=False):
    positions = np.asarray(positions, dtype=np.float32)
    colors = np.asarray(colors, dtype=np.float32)
    opacities = np.asarray(opacities, dtype=np.float32)
    scales = np.asarray(scales, dtype=np.float32)
    qvec = np.asarray(qvec, dtype=np.float32)
    tvec = np.asarray(tvec, dtype=np.float32)
    tile_hw = int(tile_hw)
    chunk_gauss = int(chunk_gauss)
    n = positions.shape[0]
    assert n == N_GAUSS, f"expected {N_GAUSS} gaussians, got {n}"

    # ---- O(N) per-gaussian prep in float64 (rounds to the same f32 values
    # the reference computes, to well within the exp's own error budget) ----
    R = _quat2mat(qvec.astype(np.float64))
    cam = positions.astype(np.float64) @ R.T + tvec.astype(np.float64)
    ax = cam[:, 0] / cam[:, 2] * FX + CX          # [N] screen x center
    ay = cam[:, 1] / cam[:, 2] * FY + CY          # [N] screen y center
    var = scales[:, 0].astype(np.float64) ** 2
    s = -0.5 / var                                # [N] negative inv 2*var

    # centered coords keep the quadratic-expansion terms small (|u|<=64)
    dx = ax - CX
    dy = ay - CY

    def f32r_round(x):
        """Round to the f32r grid (low 12 mantissa bits of fp32 cleared)."""
        v32 = np.asarray(x, dtype=np.float32).view(np.uint32)
        return ((v32 + 0x800) & np.uint32(0xFFFFF000)).view(np.float32)

    def hilo(x):
        """Split x into f32r-representable hi+lo with hi+lo ~= x to ~2^-24."""
        hi = f32r_round(x).astype(np.float64)
        lo = f32r_round(np.asarray(x, dtype=np.float64) - hi)
        return hi.astype(np.float32), lo.astype(np.float32)

    # K=12 stationary rows per gaussian (hi/lo pairs), for
    #   arg_x = s*u^2 + (-2 s dx)*u + s*dx^2     (u = x - 64)
    #   arg_y = s*v^2 + (-2 s dy)*v + s*dy^2     (v = y - 64)
    # u^2 <= 4096 is exact in f32r (12-bit significand), so hi-row products
    # are exact in the PE and lo rows mop up the residue: the f32r arg
    # matmul matches fp32 to ~1e-6 despite the quadratic cancellation.
    rows6 = [s, -2.0 * s * dx, s * dx * dx,
             s, -2.0 * s * dy, s * dy * dy]
    coef_rows = []
    for r in rows6:
        hi, lo = hilo(r)
        coef_rows.extend([hi, lo])
    coef_full = np.stack(coef_rows).astype(np.float32)   # [12, N]

    u = np.arange(W, dtype=np.float64) - CX
    v = np.arange(H, dtype=np.float64) - CY
    zeros = np.zeros(128)
    ones = np.ones(128)
    rhs_rows = []
    for base in (u * u, u, ones):
        row = np.concatenate([base, zeros]).astype(np.float32)
        rhs_rows.extend([row, row])   # hi and lo coef rows share the base
    for base in (v * v, v, ones):
        row = np.concatenate([zeros, base]).astype(np.float32)
        rhs_rows.extend([row, row])
    rhs6 = np.stack(rhs_rows)                             # [12, 256]
    if USE_PACK:
        # replicate at partition bands 0/32/64/96 for the row-group packing
        rhsxy = np.zeros((128, ARG_W), dtype=np.float32)
        for k in range(PACK):
            rhsxy[32 * k:32 * k + KARG] = rhs6
    else:
        rhsxy = rhs6

    # [N, 4] = [r, g, b, op]: op goes into B = op*expy; colors multiply the
    # already-rounded B (see kernel comment on fp16 rounding cancellation)
    op = opacities[:, 0].astype(np.float64)
    opc_full = np.concatenate(
        [colors.astype(np.float64), op[:, None]], axis=1
    ).astype(np.float32)

    # ---- shard gaussians across the 8 cores ----
    in_maps = []
    for core in range(N_CORES):
        g0 = core * G_PER_CORE
        g1 = g0 + G_PER_CORE
        opc_c = opc_full[g0:g1].reshape(N_CHUNKS, CHUNK, 4)
        opc_c = np.ascontiguousarray(
            opc_c.transpose(1, 0, 2).reshape(CHUNK, N_CHUNKS * 4))
        if USE_PACK:
            # coefpack[32k+r, grp*128+j] = coef row r of chunk grp*PACK+k
            cc = coef_full[:, g0:g1].reshape(KARG, N_GROUPS, PACK, CHUNK)
            coefpack = np.zeros((128, N_GROUPS * CHUNK), dtype=np.float32)
            for k in range(PACK):
                coefpack[32 * k:32 * k + KARG] = (
                    cc[:, :, k, :].reshape(KARG, N_GROUPS * CHUNK))
        else:
            coefpack = np.ascontiguousarray(coef_full[:, g0:g1])
        in_maps.append({
            "coef": coefpack,
            "rhsxy": rhsxy,
            "opc": opc_c,
        })

    nc = _get_program()
    res = run_bass_kernel_spmd(nc, in_maps, list(range(N_CORES)),
                               trace=_trace)

    # ---- host reduction: sum per-core partials, divide, reshape ----
    acc = np.zeros((128, 4, 128), dtype=np.float64)   # [x, c, y]
    for core in range(N_CORES):
        acc += res.results[core]["out"].reshape(128, 4, 128)

    num = acc[:, 0:3, :]                          # [x, c, y]
    n_chunks_ref = n // chunk_gauss
    den = acc[:, 3, :] + n_chunks_ref * EPS       # [x, y]
    img = num / den[:, None, :]                   # [x, c, y]
    img = img.transpose(2, 0, 1).reshape(H * W, 3)  # [p=(y,x), c]

    step = tile_hw * tile_hw
    t = (H * W) // step
    out = img.reshape(t, step, 3).transpose(0, 2, 1).reshape(
        t, 3, tile_hw, tile_hw)
    result = out.astype(np.float32)
    if _trace:
        return result, res
    return result


# revision 25
# speedup vs baseline: 2.6867x; 1.0181x over previous
"""Trainium2 Bass kernel for the isotropic-gaussian differentiable renderer.

Math: for pixel p=(x,y) and gaussian g:
    w[g,p] = op_g * exp(-0.5*((x-ax_g)^2+(y-ay_g)^2)/var_g)
    img[p,c] = (sum_g w[g,p]*col_gc) / (sum_g w[g,p] + n_chunks*EPS)

The isotropic RBF is separable: w = op * exp(sx) * exp(sy) with
sx = s*(x-ax)^2, sy = s*(y-ay)^2, s = -0.5/var.  That turns the
268M-element exp into 2*N*128 exps plus matmuls:

  per 128-gaussian chunk:
    PE (fp32): arg[g, 0:128]=sx(g,x), arg[g,128:256]=sy(g,y) via a K=6
               matmul against fixed rows [u^2,u,1|v^2,v,1] (centered coords;
               fp32 needed: the expansion cancels catastrophically)
    ACT      : expxy = exp(arg)  (PSUM->SBUF, batched over chunks)
    DVE      : A[g, c*128+y] = opc[g,c]*expy[g,y]   (4 tensor_scalar ops)
    PE       : acc[x, c*128+y] += expx^T @ A        (accumulated in PSUM)

Sharding: gaussians split 2048/core across 8 cores; every core accumulates
the full 128x128 image; host sums the 8 partials, divides num/den and
reshapes to the reference's [4,3,64,64] tile layout.
"""
import numpy as np

import concourse.bacc as bacc
import concourse.tile as tile
from concourse import mybir
from concourse.bass_utils import run_bass_kernel_spmd

# Problem constants (hardcoded per harness contract)
N_GAUSS = 16384
H = 128
W = 128
FX = 128.0
FY = 128.0
CX = 64.0
CY = 64.0
EPS = 1e-8
N_CORES = 8
G_PER_CORE = N_GAUSS // N_CORES      # 2048
CHUNK = 128                          # gaussians per matmul chunk
N_CHUNKS = G_PER_CORE // CHUNK       # 16
ARG_W = 256                          # per-chunk arg width: 128 x | 128 y
GROUP = 4                            # chunks per exp batch
N_GROUPS = N_CHUNKS // GROUP         # 4
OUT_W = 512                          # (c,y) free width of the accumulator

F32 = mybir.dt.float32
MM_DT = mybir.dt.float16             # main-accumulation matmul dtype.
# fp16 is safe here because of how A is factored: B = op*expy is rounded
# once and BOTH num and den consume the same rounded B (and the same
# rounded expx), so weight-rounding cancels in num/den; only the color
# weights carry an independent 2^-11 rounding, which averages out.
F32R = mybir.dt.float32r
KARG = 12                            # arg-matmul contraction: 6 coef rows x hi/lo
PACK = 4                             # arg matmuls packed per PE pass (row groups)
USE_PACK = False                     # fp32 + tile_position hangs TRN2; keep off


def build_program():
    """One SPMD Bass program; every core runs it on its gaussian slice."""
    nc = bacc.Bacc("TRN2", target_bir_lowering=False, debug=False,
                   num_devices=N_CORES)
    # packed: [128, 4*128]: coefpack[32k+r, grp*128+j] = coef row r of chunk
    # (grp*PACK+k), gaussian j — four chunks stacked at partition 0/32/64/96
    # so four K=6 arg matmuls run concurrently in separate PE row groups.
    # unpacked: [6, 2048] flat, one chunk per 128 columns.
    coef_shape = [128, N_GROUPS * CHUNK] if USE_PACK else [KARG, G_PER_CORE]
    coef = nc.dram_tensor("coef", coef_shape, F32, kind="ExternalInput")
    # the 6 fixed moving rows [u^2,u,1|0] / [0|v^2,v,1] (replicated at
    # partition bands 0/32/64/96 when packed).
    rhs_shape = [128, ARG_W] if USE_PACK else [KARG, ARG_W]
    rhsxy = nc.dram_tensor("rhsxy", rhs_shape, F32, kind="ExternalInput")
    # [128, 64]: opc[p, chunk*4+c] = (op*[r,g,b,1])[chunk*128+p, c]
    opc = nc.dram_tensor("opc", [128, N_CHUNKS * 4], F32, kind="ExternalInput")
    # partial accumulator: [x, c*128+y]
    out = nc.dram_tensor("out", [128, OUT_W], F32, kind="ExternalOutput")

    with tile.TileContext(nc) as tc:
        with tc.tile_pool(name="ins", bufs=1) as ins_pool, \
             tc.tile_pool(name="expp", bufs=1) as exp_pool, \
             tc.tile_pool(name="apool", bufs=3) as a_pool, \
             tc.tile_pool(name="args", bufs=2, space="PSUM") as arg_pool, \
             tc.tile_pool(name="acc", bufs=1, space="PSUM") as acc_pool, \
             tc.tile_pool(name="outp", bufs=1) as out_pool:

            coef_t = ins_pool.tile(coef_shape, F32)
            rhs_t = ins_pool.tile(rhs_shape, F32)
            opc_t = ins_pool.tile([128, N_CHUNKS * 4], F32)
            # parallel triggers spread across engine queues; coef split by
            # group so group 0's arg matmuls start as soon as possible
            GW = N_GROUPS * CHUNK  # columns per coef group slice
            nc.scalar.dma_start(out=rhs_t, in_=rhsxy[:, :])
            nc.sync.dma_start(out=coef_t[:, 0 * GW:1 * GW], in_=coef[:, 0 * GW:1 * GW])
            nc.scalar.dma_start(out=coef_t[:, 1 * GW:2 * GW], in_=coef[:, 1 * GW:2 * GW])
            nc.sync.dma_start(out=coef_t[:, 2 * GW:3 * GW], in_=coef[:, 2 * GW:3 * GW])
            nc.scalar.dma_start(out=coef_t[:, 3 * GW:4 * GW], in_=coef[:, 3 * GW:4 * GW])
            nc.gpsimd.dma_start(out=opc_t, in_=opc[:, :])

            # f32r operands must be produced by an on-chip rounding op; the
            # host pre-rounds to the f32r grid so these casts are exact.
            coef_r = ins_pool.tile(coef_shape, F32R)
            rhs_r = ins_pool.tile(rhs_shape, F32R)
            nc.vector.tensor_copy(rhs_r, rhs_t)
            for g in range(N_GROUPS):
                nc.vector.tensor_copy(coef_r[:, g * GW:(g + 1) * GW],
                                      coef_t[:, g * GW:(g + 1) * GW])

            # exp(arg) results for all chunks: [g_part, chunk*256 + (x|y)]
            expxy = exp_pool.tile([128, N_CHUNKS * ARG_W], MM_DT)
            acc = acc_pool.tile([128, OUT_W], F32)

            # PE warmup: ~2.5us of dummy matmuls during the DMA window holds
            # the HAM clock-gate at 8/8 so the real matmuls run at 2.4 GHz.
            warm = arg_pool.tile([128, ARG_W], F32, tag="warm")
            for _ in range(8):
                nc.tensor.matmul(warm[:, :], rhs_r[:, :CHUNK], rhs_r[:, :],
                                 start=True, stop=True)

            for grp in range(N_GROUPS):
                args = arg_pool.tile([128, GROUP * ARG_W], F32, tag="args")
                for k in range(PACK):
                    chunk = grp * PACK + k
                    if USE_PACK:
                        bp = 32 * k
                        lhsT = coef_r[bp:bp + KARG,
                                      grp * CHUNK:(grp + 1) * CHUNK]
                        rhs = rhs_r[bp:bp + KARG, :]
                        tp = (bp, 0)
                    else:
                        lhsT = coef_r[:, chunk * CHUNK:(chunk + 1) * CHUNK]
                        rhs = rhs_r[:, :]
                        tp = None
                    nc.tensor.matmul(
                        args[:, k * ARG_W:(k + 1) * ARG_W],
                        lhsT, rhs,
                        start=True, stop=True,
                        tile_position=tp,
                    )
                nc.scalar.activation(
                    out=expxy[:, grp * GROUP * ARG_W:(grp + 1) * GROUP * ARG_W],
                    in_=args[:, :],
                    func=mybir.ActivationFunctionType.Exp,
                )

            for chunk in range(N_CHUNKS):
                ex0 = chunk * ARG_W
                a_t = a_pool.tile([128, OUT_W], MM_DT, tag="a")
                # B = op*expy into the den column block, then the color
                # blocks from the ROUNDED B so num/den rounding cancels.
                nc.vector.tensor_scalar_mul(
                    out=a_t[:, 384:512],
                    in0=expxy[:, ex0 + 128:ex0 + 256],
                    scalar1=opc_t[:, chunk * 4 + 3:chunk * 4 + 4],
                )
                for c in range(3):
                    nc.vector.tensor_scalar_mul(
                        out=a_t[:, c * 128:(c + 1) * 128],
                        in0=a_t[:, 384:512],
                        scalar1=opc_t[:, chunk * 4 + c:chunk * 4 + c + 1],
                    )
                nc.tensor.matmul(
                    acc[:, :],
                    expxy[:, ex0:ex0 + 128],
                    a_t[:, :],
                    start=(chunk == 0), stop=(chunk == N_CHUNKS - 1),
                )

            out_t = out_pool.tile([128, OUT_W], F32)
            nc.scalar.copy(out=out_t, in_=acc)
            nc.sync.dma_start(out=out[:, :256], in_=out_t[:, :256])
            nc.scalar.dma_start(out=out[:, 256:], in_=out_t[:, 256:])

    nc.compile()
    return nc


_PROGRAM = None


def _get_program():
    global _PROGRAM
    if _PROGRAM is None:
        _PROGRAM = build_program()
    return _PROGRAM


def _quat2mat(q):
    q = q / np.linalg.norm(q)
    w, x, y, z = q
    return np.array([
        [1 - 2 * (y * y + z * z), 2 * (x * y - z * w), 2 * (x * z + y * w)],
        [2 * (x * y + z * w), 1 - 2 * (x * x + z * z), 2 * (y * z - x * w)],
        [2 * (x * z - y * w), 2 * (y * z + x * w), 1 - 2 * (x * x + y * y)],
    ])


def kernel(positions, colors, opacities, scales, qvec, tvec, tile_hw,
           chunk_gauss, _trace=False):
    positions = np.asarray(positions, dtype=np.float32)
    colors = np.asarray(colors, dtype=np.float32)
    opacities = np.asarray(opacities, dtype=np.float32)
    scales = np.asarray(scales, dtype=np.float32)
    qvec = np.asarray(qvec, dtype=np.float32)
    tvec = np.asarray(tvec, dtype=np.float32)
    tile_hw = int(tile_hw)
    chunk_gauss = int(chunk_gauss)
    n = positions.shape[0]
    assert n == N_GAUSS, f"expected {N_GAUSS} gaussians, got {n}"

    # ---- O(N) per-gaussian prep in float64 (rounds to the same f32 values
    # the reference computes, to well within the exp's own error budget) ----
    R = _quat2mat(qvec.astype(np.float64))
    cam = positions.astype(np.float64) @ R.T + tvec.astype(np.float64)
    ax = cam[:, 0] / cam[:, 2] * FX + CX          # [N] screen x center
    ay = cam[:, 1] / cam[:, 2] * FY + CY          # [N] screen y center
    var = scales[:, 0].astype(np.float64) ** 2
    s = -0.5 / var                                # [N] negative inv 2*var

    # centered coords keep the quadratic-expansion terms small (|u|<=64)
    dx = ax - CX
    dy = ay - CY

    def f32r_round(x):
        """Round to the f32r grid (low 12 mantissa bits of fp32 cleared)."""
        v32 = np.asarray(x, dtype=np.float32).view(np.uint32)
        return ((v32 + 0x800) & np.uint32(0xFFFFF000)).view(np.float32)

    def hilo(x):
        """Split x into f32r-representable hi+lo with hi+lo ~= x to ~2^-24."""
        hi = f32r_round(x).astype(np.float64)
        lo = f32r_round(np.asarray(x, dtype=np.float64) - hi)
        return hi.astype(np.float32), lo.astype(np.float32)

    # K=12 stationary rows per gaussian (hi/lo pairs), for
    #   arg_x = s*u^2 + (-2 s dx)*u + s*dx^2     (u = x - 64)
    #   arg_y = s*v^2 + (-2 s dy)*v + s*dy^2     (v = y - 64)
    # u^2 <= 4096 is exact in f32r (12-bit significand), so hi-row products
    # are exact in the PE and lo rows mop up the residue: the f32r arg
    # matmul matches fp32 to ~1e-6 despite the quadratic cancellation.
    rows6 = [s, -2.0 * s * dx, s * dx * dx,
             s, -2.0 * s * dy, s * dy * dy]
    coef_rows = []
    for r in rows6:
        hi, lo = hilo(r)
        coef_rows.extend([hi, lo])
    coef_full = np.stack(coef_rows).astype(np.float32)   # [12, N]

    u = np.arange(W, dtype=np.float64) - CX
    v = np.arange(H, dtype=np.float64) - CY
    zeros = np.zeros(128)
    ones = np.ones(128)
    rhs_rows = []
    for base in (u * u, u, ones):
        row = np.concatenate([base, zeros]).astype(np.float32)
        rhs_rows.extend([row, row])   # hi and lo coef rows share the base
    for base in (v * v, v, ones):
        row = np.concatenate([zeros, base]).astype(np.float32)
        rhs_rows.extend([row, row])
    rhs6 = np.stack(rhs_rows)                             # [12, 256]
    if USE_PACK:
        # replicate at partition bands 0/32/64/96 for the row-group packing
        rhsxy = np.zeros((128, ARG_W), dtype=np.float32)
        for k in range(PACK):
            rhsxy[32 * k:32 * k + KARG] = rhs6
    else:
        rhsxy = rhs6

    # [N, 4] = [r, g, b, op]: op goes into B = op*expy; colors multiply the
    # already-rounded B (see kernel comment on fp16 rounding cancellation)
    op = opacities[:, 0].astype(np.float64)
    opc_full = np.concatenate(
        [colors.astype(np.float64), op[:, None]], axis=1
    ).astype(np.float32)

    # ---- shard gaussians across the 8 cores ----
    in_maps = []
    for core in range(N_CORES):
        g0 = core * G_PER_CORE
        g1 = g0 + G_PER_CORE
        opc_c = opc_full[g0:g1].reshape(N_CHUNKS, CHUNK, 4)
        opc_c = np.ascontiguousarray(
            opc_c.transpose(1, 0, 2).reshape(CHUNK, N_CHUNKS * 4))
        if USE_PACK:
            # coefpack[32k+r, grp*128+j] = coef row r of chunk grp*PACK+k
            cc = coef_full[:, g0:g1].reshape(KARG, N_GROUPS, PACK, CHUNK)
            coefpack = np.zeros((128, N_GROUPS * CHUNK), dtype=np.float32)
            for k in range(PACK):
                coefpack[32 * k:32 * k + KARG] = (
                    cc[:, :, k, :].reshape(KARG, N_GROUPS * CHUNK))
        else:
            coefpack = np.ascontiguousarray(coef_full[:, g0:g1])
        in_maps.append({
            "coef": coefpack,
            "rhsxy": rhsxy,
            "opc": opc_c,
        })

    nc = _get_program()
    res = run_bass_kernel_spmd(nc, in_maps, list(range(N_CORES)),
                               trace=_trace)

    # ---- host reduction: sum per-core partials, divide, reshape ----
    acc = np.zeros((128, 4, 128), dtype=np.float64)   # [x, c, y]
    for core in range(N_CORES):
        acc += res.results[core]["out"].reshape(128, 4, 128)

    num = acc[:, 0:3, :]                          # [x, c, y]
    n_chunks_ref = n // chunk_gauss
    den = acc[:, 3, :] + n_chunks_ref * EPS       # [x, y]
    img = num / den[:, None, :]                   # [x, c, y]
    img = img.transpose(2, 0, 1).reshape(H * W, 3)  # [p=(y,x), c]

    step = tile_hw * tile_hw
    t = (H * W) // step
    out = img.reshape(t, step, 3).transpose(0, 2, 1).reshape(
        t, 3, tile_hw, tile_hw)
    result = out.astype(np.float32)
    if _trace:
        return result, res
    return result


# revision 31
# speedup vs baseline: 2.7133x; 1.0099x over previous
"""Trainium2 Bass kernel for the isotropic-gaussian differentiable renderer.

Math: for pixel p=(x,y) and gaussian g:
    w[g,p] = op_g * exp(-0.5*((x-ax_g)^2+(y-ay_g)^2)/var_g)
    img[p,c] = (sum_g w[g,p]*col_gc) / (sum_g w[g,p] + n_chunks*EPS)

The isotropic RBF is separable: w = op * exp(sx) * exp(sy) with
sx = s*(x-ax)^2, sy = s*(y-ay)^2, s = -0.5/var.  That turns the
268M-element exp into 2*N*128 exps plus matmuls:

  per 128-gaussian chunk:
    PE (fp32): arg[g, 0:128]=sx(g,x), arg[g,128:256]=sy(g,y) via a K=6
               matmul against fixed rows [u^2,u,1|v^2,v,1] (centered coords;
               fp32 needed: the expansion cancels catastrophically)
    ACT      : expxy = exp(arg)  (PSUM->SBUF, batched over chunks)
    DVE      : A[g, c*128+y] = opc[g,c]*expy[g,y]   (4 tensor_scalar ops)
    PE       : acc[x, c*128+y] += expx^T @ A        (accumulated in PSUM)

Sharding: gaussians split 2048/core across 8 cores; every core accumulates
the full 128x128 image; host sums the 8 partials, divides num/den and
reshapes to the reference's [4,3,64,64] tile layout.
"""
import numpy as np

import concourse.bacc as bacc
import concourse.tile as tile
from concourse import mybir
from concourse.bass_utils import run_bass_kernel_spmd

# Problem constants (hardcoded per harness contract)
N_GAUSS = 16384
H = 128
W = 128
FX = 128.0
FY = 128.0
CX = 64.0
CY = 64.0
EPS = 1e-8
N_CORES = 8
G_PER_CORE = N_GAUSS // N_CORES      # 2048
CHUNK = 128                          # gaussians per matmul chunk
N_CHUNKS = G_PER_CORE // CHUNK       # 16
ARG_W = 256                          # per-chunk arg width: 128 x | 128 y
GROUP = 4                            # chunks per exp batch
N_GROUPS = N_CHUNKS // GROUP         # 4
OUT_W = 512                          # (c,y) free width of the accumulator

F32 = mybir.dt.float32
MM_DT = mybir.dt.float16             # main-accumulation matmul dtype.
# fp16 is safe here because of how A is factored: B = op*expy is rounded
# once and BOTH num and den consume the same rounded B (and the same
# rounded expx), so weight-rounding cancels in num/den; only the color
# weights carry an independent 2^-11 rounding, which averages out.
F32R = mybir.dt.float32r
KARG = 12                            # arg-matmul contraction: 6 coef rows x hi/lo
PACK = 4                             # arg matmuls packed per PE pass (row groups)
USE_PACK = False                     # fp32 + tile_position hangs TRN2; keep off


def build_program():
    """One SPMD Bass program; every core runs it on its gaussian slice."""
    nc = bacc.Bacc("TRN2", target_bir_lowering=False, debug=False,
                   num_devices=N_CORES)
    # packed: [128, 4*128]: coefpack[32k+r, grp*128+j] = coef row r of chunk
    # (grp*PACK+k), gaussian j — four chunks stacked at partition 0/32/64/96
    # so four K=6 arg matmuls run concurrently in separate PE row groups.
    # unpacked: [6, 2048] flat, one chunk per 128 columns.
    coef_shape = [128, N_GROUPS * CHUNK] if USE_PACK else [KARG, G_PER_CORE]
    coef = nc.dram_tensor("coef", coef_shape, F32, kind="ExternalInput")
    # the 6 fixed moving rows [u^2,u,1|0] / [0|v^2,v,1] (replicated at
    # partition bands 0/32/64/96 when packed).
    rhs_shape = [128, ARG_W] if USE_PACK else [KARG, ARG_W]
    rhsxy = nc.dram_tensor("rhsxy", rhs_shape, F32, kind="ExternalInput")
    # [128, 64]: opc[p, chunk*4+c] = (op*[r,g,b,1])[chunk*128+p, c]
    opc = nc.dram_tensor("opc", [128, N_CHUNKS * 4], F32, kind="ExternalInput")
    # partial accumulator: [x, c*128+y]
    out = nc.dram_tensor("out", [128, OUT_W], F32, kind="ExternalOutput")

    with tile.TileContext(nc) as tc:
        with tc.tile_pool(name="ins", bufs=1) as ins_pool, \
             tc.tile_pool(name="expp", bufs=1) as exp_pool, \
             tc.tile_pool(name="apool", bufs=3) as a_pool, \
             tc.tile_pool(name="args", bufs=2, space="PSUM") as arg_pool, \
             tc.tile_pool(name="acc", bufs=1, space="PSUM") as acc_pool, \
             tc.tile_pool(name="outp", bufs=1) as out_pool:

            coef_t = ins_pool.tile(coef_shape, F32)
            rhs_t = ins_pool.tile(rhs_shape, F32)
            opc_t = ins_pool.tile([128, N_CHUNKS * 4], F32)
            # parallel triggers spread across engine queues; coef split by
            # group so group 0's arg matmuls start as soon as possible
            GW = N_GROUPS * CHUNK  # columns per coef group slice
            nc.scalar.dma_start(out=rhs_t, in_=rhsxy[:, :])
            nc.sync.dma_start(out=coef_t[:, 0 * GW:1 * GW], in_=coef[:, 0 * GW:1 * GW])
            nc.scalar.dma_start(out=coef_t[:, 1 * GW:2 * GW], in_=coef[:, 1 * GW:2 * GW])
            nc.sync.dma_start(out=coef_t[:, 2 * GW:3 * GW], in_=coef[:, 2 * GW:3 * GW])
            nc.scalar.dma_start(out=coef_t[:, 3 * GW:4 * GW], in_=coef[:, 3 * GW:4 * GW])
            nc.gpsimd.dma_start(out=opc_t, in_=opc[:, :])

            # f32r operands must be produced by an on-chip rounding op; the
            # host pre-rounds to the f32r grid so these casts are exact.
            # Run the casts on ScalarE (idle until the first exp) to keep
            # the Vector engine free for the A-build.
            coef_r = ins_pool.tile(coef_shape, F32R)
            rhs_r = ins_pool.tile(rhs_shape, F32R)
            nc.scalar.copy(out=rhs_r, in_=rhs_t)
            for g in range(N_GROUPS):
                nc.scalar.copy(out=coef_r[:, g * GW:(g + 1) * GW],
                               in_=coef_t[:, g * GW:(g + 1) * GW])

            # exp(arg) results for all chunks: [g_part, chunk*256 + (x|y)]
            expxy = exp_pool.tile([128, N_CHUNKS * ARG_W], MM_DT)
            acc = acc_pool.tile([128, OUT_W], F32)

            for grp in range(N_GROUPS):
                args = arg_pool.tile([128, GROUP * ARG_W], F32, tag="args")
                for k in range(PACK):
                    chunk = grp * PACK + k
                    if USE_PACK:
                        bp = 32 * k
                        lhsT = coef_r[bp:bp + KARG,
                                      grp * CHUNK:(grp + 1) * CHUNK]
                        rhs = rhs_r[bp:bp + KARG, :]
                        tp = (bp, 0)
                    else:
                        lhsT = coef_r[:, chunk * CHUNK:(chunk + 1) * CHUNK]
                        rhs = rhs_r[:, :]
                        tp = None
                    nc.tensor.matmul(
                        args[:, k * ARG_W:(k + 1) * ARG_W],
                        lhsT, rhs,
                        start=True, stop=True,
                        tile_position=tp,
                    )
                nc.scalar.activation(
                    out=expxy[:, grp * GROUP * ARG_W:(grp + 1) * GROUP * ARG_W],
                    in_=args[:, :],
                    func=mybir.ActivationFunctionType.Exp,
                )

            for chunk in range(N_CHUNKS):
                ex0 = chunk * ARG_W
                a_t = a_pool.tile([128, OUT_W], MM_DT, tag="a")
                # The y-arg carries +ln(op), so expxy's y half IS
                # B = op*expy already. Copy it into the den block and build
                # the color blocks from the SAME rounded B so num/den
                # rounding cancels.
                nc.vector.tensor_copy(
                    a_t[:, 384:512], expxy[:, ex0 + 128:ex0 + 256])
                for c in range(3):
                    nc.vector.tensor_scalar_mul(
                        out=a_t[:, c * 128:(c + 1) * 128],
                        in0=expxy[:, ex0 + 128:ex0 + 256],
                        scalar1=opc_t[:, chunk * 4 + c:chunk * 4 + c + 1],
                    )
                nc.tensor.matmul(
                    acc[:, :],
                    expxy[:, ex0:ex0 + 128],
                    a_t[:, :],
                    start=(chunk == 0), stop=(chunk == N_CHUNKS - 1),
                )

            out_t = out_pool.tile([128, OUT_W], F32)
            nc.scalar.copy(out=out_t[:, :256], in_=acc[:, :256])
            nc.scalar.dma_start(out=out[:, :256], in_=out_t[:, :256])
            nc.scalar.copy(out=out_t[:, 256:], in_=acc[:, 256:])
            nc.scalar.dma_start(out=out[:, 256:], in_=out_t[:, 256:])

    nc.compile()
    return nc


_PROGRAM = None


def _get_program():
    global _PROGRAM
    if _PROGRAM is None:
        _PROGRAM = build_program()
    return _PROGRAM


def _quat2mat(q):
    q = q / np.linalg.norm(q)
    w, x, y, z = q
    return np.array([
        [1 - 2 * (y * y + z * z), 2 * (x * y - z * w), 2 * (x * z + y * w)],
        [2 * (x * y + z * w), 1 - 2 * (x * x + z * z), 2 * (y * z - x * w)],
        [2 * (x * z - y * w), 2 * (y * z + x * w), 1 - 2 * (x * x + y * y)],
    ])


def kernel(positions, colors, opacities, scales, qvec, tvec, tile_hw,
           chunk_gauss, _trace=False):
    positions = np.asarray(positions, dtype=np.float32)
    colors = np.asarray(colors, dtype=np.float32)
    opacities = np.asarray(opacities, dtype=np.float32)
    scales = np.asarray(scales, dtype=np.float32)
    qvec = np.asarray(qvec, dtype=np.float32)
    tvec = np.asarray(tvec, dtype=np.float32)
    tile_hw = int(tile_hw)
    chunk_gauss = int(chunk_gauss)
    n = positions.shape[0]
    assert n == N_GAUSS, f"expected {N_GAUSS} gaussians, got {n}"

    # ---- O(N) per-gaussian prep in float64 (rounds to the same f32 values
    # the reference computes, to well within the exp's own error budget) ----
    R = _quat2mat(qvec.astype(np.float64))
    cam = positions.astype(np.float64) @ R.T + tvec.astype(np.float64)
    ax = cam[:, 0] / cam[:, 2] * FX + CX          # [N] screen x center
    ay = cam[:, 1] / cam[:, 2] * FY + CY          # [N] screen y center
    var = scales[:, 0].astype(np.float64) ** 2
    s = -0.5 / var                                # [N] negative inv 2*var

    # centered coords keep the quadratic-expansion terms small (|u|<=64)
    dx = ax - CX
    dy = ay - CY

    def f32r_round(x):
        """Round to the f32r grid (low 12 mantissa bits of fp32 cleared)."""
        v32 = np.asarray(x, dtype=np.float32).view(np.uint32)
        return ((v32 + 0x800) & np.uint32(0xFFFFF000)).view(np.float32)

    def hilo(x):
        """Split x into f32r-representable hi+lo with hi+lo ~= x to ~2^-24."""
        hi = f32r_round(x).astype(np.float64)
        lo = f32r_round(np.asarray(x, dtype=np.float64) - hi)
        return hi.astype(np.float32), lo.astype(np.float32)

    # K=12 stationary rows per gaussian (hi/lo pairs), for
    #   arg_x = s*u^2 + (-2 s dx)*u + s*dx^2     (u = x - 64)
    #   arg_y = s*v^2 + (-2 s dy)*v + s*dy^2     (v = y - 64)
    # u^2 <= 4096 is exact in f32r (12-bit significand), so hi-row products
    # are exact in the PE and lo rows mop up the residue: the f32r arg
    # matmul matches fp32 to ~1e-6 despite the quadratic cancellation.
    # +ln(op) on the y-constant row makes exp(arg_y) = op*exp_y directly
    op64 = opacities[:, 0].astype(np.float64)
    rows6 = [s, -2.0 * s * dx, s * dx * dx,
             s, -2.0 * s * dy, s * dy * dy + np.log(op64)]
    coef_rows = []
    for r in rows6:
        hi, lo = hilo(r)
        coef_rows.extend([hi, lo])
    coef_full = np.stack(coef_rows).astype(np.float32)   # [12, N]

    u = np.arange(W, dtype=np.float64) - CX
    v = np.arange(H, dtype=np.float64) - CY
    zeros = np.zeros(128)
    ones = np.ones(128)
    rhs_rows = []
    for base in (u * u, u, ones):
        row = np.concatenate([base, zeros]).astype(np.float32)
        rhs_rows.extend([row, row])   # hi and lo coef rows share the base
    for base in (v * v, v, ones):
        row = np.concatenate([zeros, base]).astype(np.float32)
        rhs_rows.extend([row, row])
    rhs6 = np.stack(rhs_rows)                             # [12, 256]
    if USE_PACK:
        # replicate at partition bands 0/32/64/96 for the row-group packing
        rhsxy = np.zeros((128, ARG_W), dtype=np.float32)
        for k in range(PACK):
            rhsxy[32 * k:32 * k + KARG] = rhs6
    else:
        rhsxy = rhs6

    # [N, 4] = [r, g, b, 1]: op is folded into the exp's y-argument
    opc_full = np.concatenate(
        [colors.astype(np.float64), np.ones((n, 1))], axis=1
    ).astype(np.float32)

    # ---- shard gaussians across the 8 cores ----
    in_maps = []
    for core in range(N_CORES):
        g0 = core * G_PER_CORE
        g1 = g0 + G_PER_CORE
        opc_c = opc_full[g0:g1].reshape(N_CHUNKS, CHUNK, 4)
        opc_c = np.ascontiguousarray(
            opc_c.transpose(1, 0, 2).reshape(CHUNK, N_CHUNKS * 4))
        if USE_PACK:
            # coefpack[32k+r, grp*128+j] = coef row r of chunk grp*PACK+k
            cc = coef_full[:, g0:g1].reshape(KARG, N_GROUPS, PACK, CHUNK)
            coefpack = np.zeros((128, N_GROUPS * CHUNK), dtype=np.float32)
            for k in range(PACK):
                coefpack[32 * k:32 * k + KARG] = (
                    cc[:, :, k, :].reshape(KARG, N_GROUPS * CHUNK))
        else:
            coefpack = np.ascontiguousarray(coef_full[:, g0:g1])
        in_maps.append({
            "coef": coefpack,
            "rhsxy": rhsxy,
            "opc": opc_c,
        })

    nc = _get_program()
    res = run_bass_kernel_spmd(nc, in_maps, list(range(N_CORES)),
                               trace=_trace)

    # ---- host reduction: sum per-core partials, divide, reshape ----
    acc = np.zeros((128, 4, 128), dtype=np.float64)   # [x, c, y]
    for core in range(N_CORES):
        acc += res.results[core]["out"].reshape(128, 4, 128)

    num = acc[:, 0:3, :]                          # [x, c, y]
    n_chunks_ref = n // chunk_gauss
    den = acc[:, 3, :] + n_chunks_ref * EPS       # [x, y]
    img = num / den[:, None, :]                   # [x, c, y]
    img = img.transpose(2, 0, 1).reshape(H * W, 3)  # [p=(y,x), c]

    step = tile_hw * tile_hw
    t = (H * W) // step
    out = img.reshape(t, step, 3).transpose(0, 2, 1).reshape(
        t, 3, tile_hw, tile_hw)
    result = out.astype(np.float32)
    if _trace:
        return result, res
    return result


# revision 34
# speedup vs baseline: 2.8469x; 1.0493x over previous
"""Trainium2 Bass kernel for the isotropic-gaussian differentiable renderer.

Math: for pixel p=(x,y) and gaussian g:
    w[g,p] = op_g * exp(-0.5*((x-ax_g)^2+(y-ay_g)^2)/var_g)
    img[p,c] = (sum_g w[g,p]*col_gc) / (sum_g w[g,p] + n_chunks*EPS)

The isotropic RBF is separable: w = op * exp(sx) * exp(sy) with
sx = s*(x-ax)^2, sy = s*(y-ay)^2, s = -0.5/var.  That turns the
268M-element exp into 2*N*128 exps plus matmuls:

  per 128-gaussian chunk:
    PE (fp32): arg[g, 0:128]=sx(g,x), arg[g,128:256]=sy(g,y) via a K=6
               matmul against fixed rows [u^2,u,1|v^2,v,1] (centered coords;
               fp32 needed: the expansion cancels catastrophically)
    ACT      : expxy = exp(arg)  (PSUM->SBUF, batched over chunks)
    DVE      : A[g, c*128+y] = opc[g,c]*expy[g,y]   (4 tensor_scalar ops)
    PE       : acc[x, c*128+y] += expx^T @ A        (accumulated in PSUM)

Sharding: gaussians split 2048/core across 8 cores; every core accumulates
the full 128x128 image; host sums the 8 partials, divides num/den and
reshapes to the reference's [4,3,64,64] tile layout.
"""
import numpy as np

import concourse.bacc as bacc
import concourse.tile as tile
from concourse import mybir
from concourse.bass_utils import run_bass_kernel_spmd

# Problem constants (hardcoded per harness contract)
N_GAUSS = 16384
H = 128
W = 128
FX = 128.0
FY = 128.0
CX = 64.0
CY = 64.0
EPS = 1e-8
N_CORES = 8
G_PER_CORE = N_GAUSS // N_CORES      # 2048
CHUNK = 128                          # gaussians per matmul chunk
N_CHUNKS = G_PER_CORE // CHUNK       # 16
ARG_W = 256                          # per-chunk arg width: 128 x | 128 y
GROUP = 4                            # chunks per exp batch
N_GROUPS = N_CHUNKS // GROUP         # 4
OUT_W = 512                          # (c,y) free width of the accumulator

F32 = mybir.dt.float32
MM_DT = mybir.dt.float16             # main-accumulation matmul dtype.
# fp16 is safe here because of how A is factored: B = op*expy is rounded
# once and BOTH num and den consume the same rounded B (and the same
# rounded expx), so weight-rounding cancels in num/den; only the color
# weights carry an independent 2^-11 rounding, which averages out.
F32R = mybir.dt.float32r
KARG = 12                            # arg-matmul contraction: 6 coef rows x hi/lo
PACK = 4                             # arg matmuls packed per PE pass (row groups)
USE_PACK = False                     # tile_position matmuls crash TRN2 here; keep off


def build_program():
    """One SPMD Bass program; every core runs it on its gaussian slice."""
    nc = bacc.Bacc("TRN2", target_bir_lowering=False, debug=False,
                   num_devices=N_CORES)
    # packed: [128, 4*128]: coefpack[32k+r, grp*128+j] = coef row r of chunk
    # (grp*PACK+k), gaussian j — four chunks stacked at partition 0/32/64/96
    # so four K=6 arg matmuls run concurrently in separate PE row groups.
    # unpacked: [6, 2048] flat, one chunk per 128 columns.
    coef_shape = [128, N_GROUPS * CHUNK] if USE_PACK else [KARG, G_PER_CORE]
    coef = nc.dram_tensor("coef", coef_shape, F32, kind="ExternalInput")
    # the 6 fixed moving rows [u^2,u,1|0] / [0|v^2,v,1] (replicated at
    # partition bands 0/32/64/96 when packed).
    rhs_shape = [128, ARG_W] if USE_PACK else [KARG, ARG_W]
    rhsxy = nc.dram_tensor("rhsxy", rhs_shape, F32, kind="ExternalInput")
    # [128, 64]: opc[p, chunk*4+c] = (op*[r,g,b,1])[chunk*128+p, c]
    opc = nc.dram_tensor("opc", [128, N_CHUNKS * 4], F32, kind="ExternalInput")
    # partial accumulator: [x, c*128+y]
    out = nc.dram_tensor("out", [128, OUT_W], F32, kind="ExternalOutput")

    with tile.TileContext(nc) as tc:
        with tc.tile_pool(name="ins", bufs=1) as ins_pool, \
             tc.tile_pool(name="expp", bufs=1) as exp_pool, \
             tc.tile_pool(name="apool", bufs=3) as a_pool, \
             tc.tile_pool(name="args", bufs=2, space="PSUM") as arg_pool, \
             tc.tile_pool(name="acc", bufs=1, space="PSUM") as acc_pool, \
             tc.tile_pool(name="outp", bufs=1) as out_pool:

            coef_t = ins_pool.tile(coef_shape, F32)
            rhs_t = ins_pool.tile(rhs_shape, F32)
            opc_t = ins_pool.tile([128, N_CHUNKS * 4], F32)
            # parallel triggers spread across engine queues; coef split by
            # group so group 0's arg matmuls start as soon as possible
            GW = CHUNK if USE_PACK else PACK * CHUNK  # coef cols per group
            nc.scalar.dma_start(out=rhs_t, in_=rhsxy[:, :])
            nc.sync.dma_start(out=coef_t[:, 0 * GW:1 * GW], in_=coef[:, 0 * GW:1 * GW])
            nc.scalar.dma_start(out=coef_t[:, 1 * GW:2 * GW], in_=coef[:, 1 * GW:2 * GW])
            nc.sync.dma_start(out=coef_t[:, 2 * GW:3 * GW], in_=coef[:, 2 * GW:3 * GW])
            nc.scalar.dma_start(out=coef_t[:, 3 * GW:4 * GW], in_=coef[:, 3 * GW:4 * GW])
            nc.gpsimd.dma_start(out=opc_t, in_=opc[:, :])

            # f32r operands must be produced by an on-chip rounding op; the
            # host pre-rounds to the f32r grid so these casts are exact.
            # Run the casts on ScalarE (idle until the first exp) to keep
            # the Vector engine free for the A-build.
            coef_r = ins_pool.tile(coef_shape, F32R)
            rhs_r = ins_pool.tile(rhs_shape, F32R)
            nc.vector.tensor_copy(rhs_r, rhs_t)
            for g in range(N_GROUPS):
                nc.vector.tensor_copy(coef_r[:, g * GW:(g + 1) * GW],
                                      coef_t[:, g * GW:(g + 1) * GW])

            # exp(arg) results for all chunks: [g_part, chunk*256 + (x|y)]
            expxy = exp_pool.tile([128, N_CHUNKS * ARG_W], MM_DT)
            acc = acc_pool.tile([128, OUT_W], F32)

            for grp in range(N_GROUPS):
                args = arg_pool.tile([128, GROUP * ARG_W], F32, tag="args")
                for k in range(PACK):
                    chunk = grp * PACK + k
                    if USE_PACK:
                        bp = 32 * k
                        lhsT = coef_r[bp:bp + KARG,
                                      grp * CHUNK:(grp + 1) * CHUNK]
                        rhs = rhs_r[bp:bp + KARG, :]
                        tp = (bp, 0)
                    else:
                        lhsT = coef_r[:, chunk * CHUNK:(chunk + 1) * CHUNK]
                        rhs = rhs_r[:, :]
                        tp = None
                    nc.tensor.matmul(
                        args[:, k * ARG_W:(k + 1) * ARG_W],
                        lhsT, rhs,
                        start=True, stop=True,
                        tile_position=tp,
                    )
                nc.scalar.activation(
                    out=expxy[:, grp * GROUP * ARG_W:(grp + 1) * GROUP * ARG_W],
                    in_=args[:, :],
                    func=mybir.ActivationFunctionType.Exp,
                )

            for chunk in range(N_CHUNKS):
                ex0 = chunk * ARG_W
                a_t = a_pool.tile([128, OUT_W], MM_DT, tag="a")
                # The y-arg carries +ln(op), so expxy's y half IS
                # B = op*expy already. Copy it into the den block and build
                # the color blocks from the SAME rounded B so num/den
                # rounding cancels.
                nc.vector.tensor_copy(
                    a_t[:, 384:512], expxy[:, ex0 + 128:ex0 + 256])
                for c in range(3):
                    nc.vector.tensor_scalar_mul(
                        out=a_t[:, c * 128:(c + 1) * 128],
                        in0=expxy[:, ex0 + 128:ex0 + 256],
                        scalar1=opc_t[:, chunk * 4 + c:chunk * 4 + c + 1],
                    )
                nc.tensor.matmul(
                    acc[:, :],
                    expxy[:, ex0:ex0 + 128],
                    a_t[:, :],
                    start=(chunk == 0), stop=(chunk == N_CHUNKS - 1),
                )

            out_t = out_pool.tile([128, OUT_W], F32)
            nc.scalar.copy(out=out_t[:, :256], in_=acc[:, :256])
            nc.scalar.dma_start(out=out[:, :256], in_=out_t[:, :256])
            nc.scalar.copy(out=out_t[:, 256:], in_=acc[:, 256:])
            nc.sync.dma_start(out=out[:, 256:], in_=out_t[:, 256:])

    nc.compile()
    return nc


_PROGRAM = None


def _get_program():
    global _PROGRAM
    if _PROGRAM is None:
        _PROGRAM = build_program()
    return _PROGRAM


def _quat2mat(q):
    q = q / np.linalg.norm(q)
    w, x, y, z = q
    return np.array([
        [1 - 2 * (y * y + z * z), 2 * (x * y - z * w), 2 * (x * z + y * w)],
        [2 * (x * y + z * w), 1 - 2 * (x * x + z * z), 2 * (y * z - x * w)],
        [2 * (x * z - y * w), 2 * (y * z + x * w), 1 - 2 * (x * x + y * y)],
    ])


def kernel(positions, colors, opacities, scales, qvec, tvec, tile_hw,
           chunk_gauss, _trace=False):
    positions = np.asarray(positions, dtype=np.float32)
    colors = np.asarray(colors, dtype=np.float32)
    opacities = np.asarray(opacities, dtype=np.float32)
    scales = np.asarray(scales, dtype=np.float32)
    qvec = np.asarray(qvec, dtype=np.float32)
    tvec = np.asarray(tvec, dtype=np.float32)
    tile_hw = int(tile_hw)
    chunk_gauss = int(chunk_gauss)
    n = positions.shape[0]
    assert n == N_GAUSS, f"expected {N_GAUSS} gaussians, got {n}"

    # ---- O(N) per-gaussian prep in float64 (rounds to the same f32 values
    # the reference computes, to well within the exp's own error budget) ----
    R = _quat2mat(qvec.astype(np.float64))
    cam = positions.astype(np.float64) @ R.T + tvec.astype(np.float64)
    ax = cam[:, 0] / cam[:, 2] * FX + CX          # [N] screen x center
    ay = cam[:, 1] / cam[:, 2] * FY + CY          # [N] screen y center
    var = scales[:, 0].astype(np.float64) ** 2
    s = -0.5 / var                                # [N] negative inv 2*var

    # centered coords keep the quadratic-expansion terms small (|u|<=64)
    dx = ax - CX
    dy = ay - CY

    def f32r_round(x):
        """Round to the f32r grid (low 12 mantissa bits of fp32 cleared)."""
        v32 = np.asarray(x, dtype=np.float32).view(np.uint32)
        return ((v32 + 0x800) & np.uint32(0xFFFFF000)).view(np.float32)

    def hilo(x):
        """Split x into f32r-representable hi+lo with hi+lo ~= x to ~2^-24."""
        hi = f32r_round(x).astype(np.float64)
        lo = f32r_round(np.asarray(x, dtype=np.float64) - hi)
        return hi.astype(np.float32), lo.astype(np.float32)

    # K=12 stationary rows per gaussian (hi/lo pairs), for
    #   arg_x = s*u^2 + (-2 s dx)*u + s*dx^2     (u = x - 64)
    #   arg_y = s*v^2 + (-2 s dy)*v + s*dy^2     (v = y - 64)
    # u^2 <= 4096 is exact in f32r (12-bit significand), so hi-row products
    # are exact in the PE and lo rows mop up the residue: the f32r arg
    # matmul matches fp32 to ~1e-6 despite the quadratic cancellation.
    # +ln(op) on the y-constant row makes exp(arg_y) = op*exp_y directly
    op64 = opacities[:, 0].astype(np.float64)
    rows6 = [s, -2.0 * s * dx, s * dx * dx,
             s, -2.0 * s * dy, s * dy * dy + np.log(op64)]
    coef_rows = []
    for r in rows6:
        hi, lo = hilo(r)
        coef_rows.extend([hi, lo])
    coef_full = np.stack(coef_rows).astype(np.float32)   # [12, N]

    u = np.arange(W, dtype=np.float64) - CX
    v = np.arange(H, dtype=np.float64) - CY
    zeros = np.zeros(128)
    ones = np.ones(128)
    rhs_rows = []
    for base in (u * u, u, ones):
        row = np.concatenate([base, zeros]).astype(np.float32)
        rhs_rows.extend([row, row])   # hi and lo coef rows share the base
    for base in (v * v, v, ones):
        row = np.concatenate([zeros, base]).astype(np.float32)
        rhs_rows.extend([row, row])
    rhs6 = np.stack(rhs_rows)                             # [12, 256]
    if USE_PACK:
        # replicate at partition bands 0/32/64/96 for the row-group packing
        rhsxy = np.zeros((128, ARG_W), dtype=np.float32)
        for k in range(PACK):
            rhsxy[32 * k:32 * k + KARG] = rhs6
    else:
        rhsxy = rhs6

    # [N, 4] = [r, g, b, 1]: op is folded into the exp's y-argument
    opc_full = np.concatenate(
        [colors.astype(np.float64), np.ones((n, 1))], axis=1
    ).astype(np.float32)

    # ---- shard gaussians across the 8 cores ----
    in_maps = []
    for core in range(N_CORES):
        g0 = core * G_PER_CORE
        g1 = g0 + G_PER_CORE
        opc_c = opc_full[g0:g1].reshape(N_CHUNKS, CHUNK, 4)
        opc_c = np.ascontiguousarray(
            opc_c.transpose(1, 0, 2).reshape(CHUNK, N_CHUNKS * 4))
        if USE_PACK:
            # coefpack[32k+r, grp*128+j] = coef row r of chunk grp*PACK+k
            cc = coef_full[:, g0:g1].reshape(KARG, N_GROUPS, PACK, CHUNK)
            coefpack = np.zeros((128, N_GROUPS * CHUNK), dtype=np.float32)
            for k in range(PACK):
                coefpack[32 * k:32 * k + KARG] = (
                    cc[:, :, k, :].reshape(KARG, N_GROUPS * CHUNK))
        else:
            coefpack = np.ascontiguousarray(coef_full[:, g0:g1])
        in_maps.append({
            "coef": coefpack,
            "rhsxy": rhsxy,
            "opc": opc_c,
        })

    nc = _get_program()
    res = run_bass_kernel_spmd(nc, in_maps, list(range(N_CORES)),
                               trace=_trace)

    # ---- host reduction: sum per-core partials, divide, reshape ----
    acc = np.zeros((128, 4, 128), dtype=np.float64)   # [x, c, y]
    for core in range(N_CORES):
        acc += res.results[core]["out"].reshape(128, 4, 128)

    num = acc[:, 0:3, :]                          # [x, c, y]
    n_chunks_ref = n // chunk_gauss
    den = acc[:, 3, :] + n_chunks_ref * EPS       # [x, y]
    img = num / den[:, None, :]                   # [x, c, y]
    img = img.transpose(2, 0, 1).reshape(H * W, 3)  # [p=(y,x), c]

    step = tile_hw * tile_hw
    t = (H * W) // step
    out = img.reshape(t, step, 3).transpose(0, 2, 1).reshape(
        t, 3, tile_hw, tile_hw)
    result = out.astype(np.float32)
    if _trace:
        return result, res
    return result


# revision 35
# speedup vs baseline: 2.9012x; 1.0191x over previous
"""Trainium2 Bass kernel for the isotropic-gaussian differentiable renderer.

Math: for pixel p=(x,y) and gaussian g:
    w[g,p] = op_g * exp(-0.5*((x-ax_g)^2+(y-ay_g)^2)/var_g)
    img[p,c] = (sum_g w[g,p]*col_gc) / (sum_g w[g,p] + n_chunks*EPS)

The isotropic RBF is separable: w = op * exp(sx) * exp(sy) with
sx = s*(x-ax)^2, sy = s*(y-ay)^2 + ln(op), s = -0.5/var.  That turns the
268M-element exp into 2*N*128 exps plus matmuls:

  per 128-gaussian chunk:
    PE (f32r): arg[g, 0:128]=sx(g,x), arg[g,128:256]=sy(g,y) via a K=12
               matmul against fixed rows [u^2,u,1|v^2,v,1] duplicated for a
               hi/lo coefficient split (centered coords; the split keeps the
               catastrophically-cancelling quadratic exact in f32r)
    ACT      : expxy = exp(arg) -> fp16  (PSUM->SBUF, batched 4 chunks/op);
               the y half is B = op*expy directly (ln(op) in the argument)
    DVE      : A = [col_r*B | col_g*B | col_b*B | B]  (3 tensor_scalar + copy;
               num and den share the SAME rounded B and expx, so fp16
               weight rounding cancels in the final num/den ratio)
    PE (fp16): acc[x, c*128+y] += expx^T @ A         (fp32 PSUM accumulate)

Sharding: gaussians split 2048/core across 8 cores; every core accumulates
the full 128x128 image; host sums the 8 partials, divides num/den and
reshapes to the reference's [4,3,64,64] tile layout.
"""
import numpy as np

import concourse.bacc as bacc
import concourse.tile as tile
from concourse import mybir
from concourse.bass_utils import run_bass_kernel_spmd

# Problem constants (hardcoded per harness contract)
N_GAUSS = 16384
H = 128
W = 128
FX = 128.0
FY = 128.0
CX = 64.0
CY = 64.0
EPS = 1e-8
N_CORES = 8
G_PER_CORE = N_GAUSS // N_CORES      # 2048
CHUNK = 128                          # gaussians per matmul chunk
N_CHUNKS = G_PER_CORE // CHUNK       # 16
ARG_W = 256                          # per-chunk arg width: 128 x | 128 y
GROUP = 4                            # chunks per exp batch
N_GROUPS = N_CHUNKS // GROUP         # 4
OUT_W = 512                          # (c,y) free width of the accumulator

F32 = mybir.dt.float32
MM_DT = mybir.dt.float16             # main-accumulation matmul dtype.
# fp16 is safe here because of how A is factored: B = op*expy is rounded
# once and BOTH num and den consume the same rounded B (and the same
# rounded expx), so weight-rounding cancels in num/den; only the color
# weights carry an independent 2^-11 rounding, which averages out.
F32R = mybir.dt.float32r
KARG = 12                            # arg-matmul contraction: 6 coef rows x hi/lo
PACK = 4                             # arg matmuls packed per PE pass (row groups)
USE_PACK = False                     # tile_position matmuls crash TRN2 here; keep off


def build_program():
    """One SPMD Bass program; every core runs it on its gaussian slice."""
    nc = bacc.Bacc("TRN2", target_bir_lowering=False, debug=False,
                   num_devices=N_CORES)
    # packed: [128, 4*128]: coefpack[32k+r, grp*128+j] = coef row r of chunk
    # (grp*PACK+k), gaussian j — four chunks stacked at partition 0/32/64/96
    # so four K=6 arg matmuls run concurrently in separate PE row groups.
    # unpacked: [6, 2048] flat, one chunk per 128 columns.
    coef_shape = [128, N_GROUPS * CHUNK] if USE_PACK else [KARG, G_PER_CORE]
    coef = nc.dram_tensor("coef", coef_shape, F32, kind="ExternalInput")
    # the 6 fixed moving rows [u^2,u,1|0] / [0|v^2,v,1] (replicated at
    # partition bands 0/32/64/96 when packed).
    rhs_shape = [128, ARG_W] if USE_PACK else [KARG, ARG_W]
    rhsxy = nc.dram_tensor("rhsxy", rhs_shape, F32, kind="ExternalInput")
    # [128, 64]: opc[p, chunk*4+c] = (op*[r,g,b,1])[chunk*128+p, c]
    opc = nc.dram_tensor("opc", [128, N_CHUNKS * 4], F32, kind="ExternalInput")
    # partial accumulator: [x, c*128+y]
    out = nc.dram_tensor("out", [128, OUT_W], F32, kind="ExternalOutput")

    with tile.TileContext(nc) as tc:
        with tc.tile_pool(name="ins", bufs=1) as ins_pool, \
             tc.tile_pool(name="expp", bufs=1) as exp_pool, \
             tc.tile_pool(name="apool", bufs=3) as a_pool, \
             tc.tile_pool(name="args", bufs=2, space="PSUM") as arg_pool, \
             tc.tile_pool(name="acc", bufs=1, space="PSUM") as acc_pool, \
             tc.tile_pool(name="outp", bufs=1) as out_pool:

            coef_t = ins_pool.tile(coef_shape, F32)
            rhs_t = ins_pool.tile(rhs_shape, F32)
            opc_t = ins_pool.tile([128, N_CHUNKS * 4], F32)
            # parallel triggers spread across engine queues; coef split by
            # group so group 0's arg matmuls start as soon as possible
            GW = CHUNK if USE_PACK else PACK * CHUNK  # coef cols per group
            nc.scalar.dma_start(out=rhs_t, in_=rhsxy[:, :])
            nc.sync.dma_start(out=coef_t[:, 0 * GW:1 * GW], in_=coef[:, 0 * GW:1 * GW])
            nc.scalar.dma_start(out=coef_t[:, 1 * GW:2 * GW], in_=coef[:, 1 * GW:2 * GW])
            nc.sync.dma_start(out=coef_t[:, 2 * GW:3 * GW], in_=coef[:, 2 * GW:3 * GW])
            nc.scalar.dma_start(out=coef_t[:, 3 * GW:4 * GW], in_=coef[:, 3 * GW:4 * GW])
            nc.gpsimd.dma_start(out=opc_t, in_=opc[:, :])

            # f32r operands must be produced by an on-chip rounding op; the
            # host pre-rounds to the f32r grid so these casts are exact.
            # Run the casts on ScalarE (idle until the first exp) to keep
            # the Vector engine free for the A-build.
            coef_r = ins_pool.tile(coef_shape, F32R)
            rhs_r = ins_pool.tile(rhs_shape, F32R)
            nc.vector.tensor_copy(rhs_r, rhs_t)
            for g in range(N_GROUPS):
                nc.vector.tensor_copy(coef_r[:, g * GW:(g + 1) * GW],
                                      coef_t[:, g * GW:(g + 1) * GW])

            # exp(arg) results for all chunks: [g_part, chunk*256 + (x|y)]
            expxy = exp_pool.tile([128, N_CHUNKS * ARG_W], MM_DT)
            acc = acc_pool.tile([128, OUT_W], F32)

            for grp in range(N_GROUPS):
                args = arg_pool.tile([128, GROUP * ARG_W], F32, tag="args")
                for k in range(PACK):
                    chunk = grp * PACK + k
                    if USE_PACK:
                        bp = 32 * k
                        lhsT = coef_r[bp:bp + KARG,
                                      grp * CHUNK:(grp + 1) * CHUNK]
                        rhs = rhs_r[bp:bp + KARG, :]
                        tp = (bp, 0)
                    else:
                        lhsT = coef_r[:, chunk * CHUNK:(chunk + 1) * CHUNK]
                        rhs = rhs_r[:, :]
                        tp = None
                    nc.tensor.matmul(
                        args[:, k * ARG_W:(k + 1) * ARG_W],
                        lhsT, rhs,
                        start=True, stop=True,
                        tile_position=tp,
                    )
                nc.scalar.activation(
                    out=expxy[:, grp * GROUP * ARG_W:(grp + 1) * GROUP * ARG_W],
                    in_=args[:, :],
                    func=mybir.ActivationFunctionType.Exp,
                )

            for chunk in range(N_CHUNKS):
                ex0 = chunk * ARG_W
                a_t = a_pool.tile([128, OUT_W], MM_DT, tag="a")
                # The y-arg carries +ln(op), so expxy's y half IS
                # B = op*expy already. Copy it into the den block and build
                # the color blocks from the SAME rounded B so num/den
                # rounding cancels.
                nc.vector.tensor_copy(
                    a_t[:, 384:512], expxy[:, ex0 + 128:ex0 + 256])
                for c in range(3):
                    nc.vector.tensor_scalar_mul(
                        out=a_t[:, c * 128:(c + 1) * 128],
                        in0=expxy[:, ex0 + 128:ex0 + 256],
                        scalar1=opc_t[:, chunk * 4 + c:chunk * 4 + c + 1],
                    )
                nc.tensor.matmul(
                    acc[:, :],
                    expxy[:, ex0:ex0 + 128],
                    a_t[:, :],
                    start=(chunk == 0), stop=(chunk == N_CHUNKS - 1),
                )

            out_t = out_pool.tile([128, OUT_W], F32)
            nc.scalar.copy(out=out_t[:, :256], in_=acc[:, :256])
            nc.scalar.dma_start(out=out[:, :256], in_=out_t[:, :256])
            nc.scalar.copy(out=out_t[:, 256:], in_=acc[:, 256:])
            nc.sync.dma_start(out=out[:, 256:], in_=out_t[:, 256:])

    nc.compile()
    return nc


_PROGRAM = None


def _get_program():
    global _PROGRAM
    if _PROGRAM is None:
        _PROGRAM = build_program()
    return _PROGRAM


def _quat2mat(q):
    q = q / np.linalg.norm(q)
    w, x, y, z = q
    return np.array([
        [1 - 2 * (y * y + z * z), 2 * (x * y - z * w), 2 * (x * z + y * w)],
        [2 * (x * y + z * w), 1 - 2 * (x * x + z * z), 2 * (y * z - x * w)],
        [2 * (x * z - y * w), 2 * (y * z + x * w), 1 - 2 * (x * x + y * y)],
    ])


def kernel(positions, colors, opacities, scales, qvec, tvec, tile_hw,
           chunk_gauss, _trace=False):
    positions = np.asarray(positions, dtype=np.float32)
    colors = np.asarray(colors, dtype=np.float32)
    opacities = np.asarray(opacities, dtype=np.float32)
    scales = np.asarray(scales, dtype=np.float32)
    qvec = np.asarray(qvec, dtype=np.float32)
    tvec = np.asarray(tvec, dtype=np.float32)
    tile_hw = int(tile_hw)
    chunk_gauss = int(chunk_gauss)
    n = positions.shape[0]
    assert n == N_GAUSS, f"expected {N_GAUSS} gaussians, got {n}"

    # ---- O(N) per-gaussian prep in float64 (rounds to the same f32 values
    # the reference computes, to well within the exp's own error budget) ----
    R = _quat2mat(qvec.astype(np.float64))
    cam = positions.astype(np.float64) @ R.T + tvec.astype(np.float64)
    ax = cam[:, 0] / cam[:, 2] * FX + CX          # [N] screen x center
    ay = cam[:, 1] / cam[:, 2] * FY + CY          # [N] screen y center
    var = scales[:, 0].astype(np.float64) ** 2
    s = -0.5 / var                                # [N] negative inv 2*var

    # centered coords keep the quadratic-expansion terms small (|u|<=64)
    dx = ax - CX
    dy = ay - CY

    def f32r_round(x):
        """Round to the f32r grid (low 12 mantissa bits of fp32 cleared)."""
        v32 = np.asarray(x, dtype=np.float32).view(np.uint32)
        return ((v32 + 0x800) & np.uint32(0xFFFFF000)).view(np.float32)

    def hilo(x):
        """Split x into f32r-representable hi+lo with hi+lo ~= x to ~2^-24."""
        hi = f32r_round(x).astype(np.float64)
        lo = f32r_round(np.asarray(x, dtype=np.float64) - hi)
        return hi.astype(np.float32), lo.astype(np.float32)

    # K=12 stationary rows per gaussian (hi/lo pairs), for
    #   arg_x = s*u^2 + (-2 s dx)*u + s*dx^2     (u = x - 64)
    #   arg_y = s*v^2 + (-2 s dy)*v + s*dy^2     (v = y - 64)
    # u^2 <= 4096 is exact in f32r (12-bit significand), so hi-row products
    # are exact in the PE and lo rows mop up the residue: the f32r arg
    # matmul matches fp32 to ~1e-6 despite the quadratic cancellation.
    # +ln(op) on the y-constant row makes exp(arg_y) = op*exp_y directly
    op64 = opacities[:, 0].astype(np.float64)
    rows6 = [s, -2.0 * s * dx, s * dx * dx,
             s, -2.0 * s * dy, s * dy * dy + np.log(op64)]
    coef_rows = []
    for r in rows6:
        hi, lo = hilo(r)
        coef_rows.extend([hi, lo])
    coef_full = np.stack(coef_rows).astype(np.float32)   # [12, N]

    u = np.arange(W, dtype=np.float64) - CX
    v = np.arange(H, dtype=np.float64) - CY
    zeros = np.zeros(128)
    ones = np.ones(128)
    rhs_rows = []
    for base in (u * u, u, ones):
        row = np.concatenate([base, zeros]).astype(np.float32)
        rhs_rows.extend([row, row])   # hi and lo coef rows share the base
    for base in (v * v, v, ones):
        row = np.concatenate([zeros, base]).astype(np.float32)
        rhs_rows.extend([row, row])
    rhs6 = np.stack(rhs_rows)                             # [12, 256]
    if USE_PACK:
        # replicate at partition bands 0/32/64/96 for the row-group packing
        rhsxy = np.zeros((128, ARG_W), dtype=np.float32)
        for k in range(PACK):
            rhsxy[32 * k:32 * k + KARG] = rhs6
    else:
        rhsxy = rhs6

    # [N, 4] = [r, g, b, 1]: op is folded into the exp's y-argument
    opc_full = np.concatenate(
        [colors.astype(np.float64), np.ones((n, 1))], axis=1
    ).astype(np.float32)

    # ---- shard gaussians across the 8 cores ----
    in_maps = []
    for core in range(N_CORES):
        g0 = core * G_PER_CORE
        g1 = g0 + G_PER_CORE
        opc_c = opc_full[g0:g1].reshape(N_CHUNKS, CHUNK, 4)
        opc_c = np.ascontiguousarray(
            opc_c.transpose(1, 0, 2).reshape(CHUNK, N_CHUNKS * 4))
        if USE_PACK:
            # coefpack[32k+r, grp*128+j] = coef row r of chunk grp*PACK+k
            cc = coef_full[:, g0:g1].reshape(KARG, N_GROUPS, PACK, CHUNK)
            coefpack = np.zeros((128, N_GROUPS * CHUNK), dtype=np.float32)
            for k in range(PACK):
                coefpack[32 * k:32 * k + KARG] = (
                    cc[:, :, k, :].reshape(KARG, N_GROUPS * CHUNK))
        else:
            coefpack = np.ascontiguousarray(coef_full[:, g0:g1])
        in_maps.append({
            "coef": coefpack,
            "rhsxy": rhsxy,
            "opc": opc_c,
        })

    nc = _get_program()
    res = run_bass_kernel_spmd(nc, in_maps, list(range(N_CORES)),
                               trace=_trace)

    # ---- host reduction: sum per-core partials, divide, reshape ----
    acc = np.zeros((128, 4, 128), dtype=np.float64)   # [x, c, y]
    for core in range(N_CORES):
        acc += res.results[core]["out"].reshape(128, 4, 128)

    num = acc[:, 0:3, :]                          # [x, c, y]
    n_chunks_ref = n // chunk_gauss
    den = acc[:, 3, :] + n_chunks_ref * EPS       # [x, y]
    img = num / den[:, None, :]                   # [x, c, y]
    img = img.transpose(2, 0, 1).reshape(H * W, 3)  # [p=(y,x), c]

    step = tile_hw * tile_hw
    t = (H * W) // step
    out = img.reshape(t, step, 3).transpose(0, 2, 1).reshape(
        t, 3, tile_hw, tile_hw)
    result = out.astype(np.float32)
    if _trace:
        return result, res
    return result


# revision 39
# speedup vs baseline: 3.0459x; 1.0499x over previous
"""Trainium2 Bass kernel for the isotropic-gaussian differentiable renderer.

Math: for pixel p=(x,y) and gaussian g:
    w[g,p] = op_g * exp(-0.5*((x-ax_g)^2+(y-ay_g)^2)/var_g)
    img[p,c] = (sum_g w[g,p]*col_gc) / (sum_g w[g,p] + n_chunks*EPS)

The isotropic RBF is separable: w = op * exp(sx) * exp(sy) with
sx = s*(x-ax)^2, sy = s*(y-ay)^2 + ln(op), s = -0.5/var.  That turns the
268M-element exp into 2*N*128 exps plus matmuls:

  per 128-gaussian chunk:
    PE (f32r): arg[g, 0:128]=sx(g,x), arg[g,128:256]=sy(g,y) via a K=12
               matmul against fixed rows [u^2,u,1|v^2,v,1] duplicated for a
               hi/lo coefficient split (centered coords; the split keeps the
               catastrophically-cancelling quadratic exact in f32r)
    ACT      : expxy = exp(arg) -> fp16  (PSUM->SBUF, batched 4 chunks/op);
               the y half is B = op*expy directly (ln(op) in the argument)
    DVE      : A = [col_r*B | col_g*B | col_b*B | B]  (3 tensor_scalar + copy;
               num and den share the SAME rounded B and expx, so fp16
               weight rounding cancels in the final num/den ratio)
    PE (fp16): acc[x, c*128+y] += expx^T @ A         (fp32 PSUM accumulate)

Sharding: gaussians split 2048/core across 8 cores; every core accumulates
the full 128x128 image; host sums the 8 partials, divides num/den and
reshapes to the reference's [4,3,64,64] tile layout.
"""
import numpy as np

import concourse.bacc as bacc
import concourse.tile as tile
from concourse import mybir
from concourse.bass_utils import run_bass_kernel_spmd

# Problem constants (hardcoded per harness contract)
N_GAUSS = 16384
H = 128
W = 128
FX = 128.0
FY = 128.0
CX = 64.0
CY = 64.0
EPS = 1e-8
N_CORES = 8
G_PER_CORE = N_GAUSS // N_CORES      # 2048
CHUNK = 128                          # gaussians per matmul chunk
N_CHUNKS = G_PER_CORE // CHUNK       # 16
ARG_W = 256                          # per-chunk arg width: 128 x | 128 y
GROUP = 4                            # chunks per exp batch
N_GROUPS = N_CHUNKS // GROUP         # 4
OUT_W = 512                          # (c,y) free width of the accumulator

F32 = mybir.dt.float32
MM_DT = mybir.dt.float16             # main-accumulation matmul dtype.
# fp16 is safe here because of how A is factored: B = op*expy is rounded
# once and BOTH num and den consume the same rounded B (and the same
# rounded expx), so weight-rounding cancels in num/den; only the color
# weights carry an independent 2^-11 rounding, which averages out.
F32R = mybir.dt.float32r
KARG = 12                            # arg-matmul contraction: 6 coef rows x hi/lo
PACK = 4                             # arg matmuls packed per PE pass (row groups)
USE_PACK = False                     # tile_position matmuls crash TRN2 here; keep off


def build_program():
    """One SPMD Bass program; every core runs it on its gaussian slice."""
    nc = bacc.Bacc("TRN2", target_bir_lowering=False, debug=False,
                   num_devices=N_CORES)
    # packed: [128, 4*128]: coefpack[32k+r, grp*128+j] = coef row r of chunk
    # (grp*PACK+k), gaussian j — four chunks stacked at partition 0/32/64/96
    # so four K=6 arg matmuls run concurrently in separate PE row groups.
    # unpacked: [6, 2048] flat, one chunk per 128 columns.
    coef_shape = [128, N_GROUPS * CHUNK] if USE_PACK else [KARG, G_PER_CORE]
    coef = nc.dram_tensor("coef", coef_shape, F32, kind="ExternalInput")
    # the 6 fixed moving rows [u^2,u,1|0] / [0|v^2,v,1] (replicated at
    # partition bands 0/32/64/96 when packed).
    rhs_shape = [128, ARG_W] if USE_PACK else [KARG, ARG_W]
    rhsxy = nc.dram_tensor("rhsxy", rhs_shape, F32, kind="ExternalInput")
    # [128, 64]: opc[p, chunk*4+c] = (op*[r,g,b,1])[chunk*128+p, c]
    opc = nc.dram_tensor("opc", [128, N_CHUNKS * 4], F32, kind="ExternalInput")
    # partial accumulator: [x, c*128+y]
    out = nc.dram_tensor("out", [128, OUT_W], F32, kind="ExternalOutput")

    with tile.TileContext(nc) as tc:
        with tc.tile_pool(name="ins", bufs=1) as ins_pool, \
             tc.tile_pool(name="expp", bufs=1) as exp_pool, \
             tc.tile_pool(name="args", bufs=2, space="PSUM") as arg_pool, \
             tc.tile_pool(name="acc", bufs=1, space="PSUM") as acc_pool, \
             tc.tile_pool(name="warmp", bufs=1, space="PSUM") as warm_pool, \
             tc.tile_pool(name="outp", bufs=1) as out_pool:

            coef_t = ins_pool.tile(coef_shape, F32)
            rhs_t = ins_pool.tile(rhs_shape, F32)
            opc_t = ins_pool.tile([128, N_CHUNKS * 4], F32)
            # parallel triggers spread across engine queues; coef split by
            # group so group 0's arg matmuls start as soon as possible
            GW = CHUNK if USE_PACK else PACK * CHUNK  # coef cols per group
            nc.scalar.dma_start(out=rhs_t, in_=rhsxy[:, :])
            nc.sync.dma_start(out=coef_t[:, 0 * GW:1 * GW], in_=coef[:, 0 * GW:1 * GW])
            nc.scalar.dma_start(out=coef_t[:, 1 * GW:2 * GW], in_=coef[:, 1 * GW:2 * GW])
            nc.sync.dma_start(out=coef_t[:, 2 * GW:3 * GW], in_=coef[:, 2 * GW:3 * GW])
            nc.scalar.dma_start(out=coef_t[:, 3 * GW:4 * GW], in_=coef[:, 3 * GW:4 * GW])
            nc.gpsimd.dma_start(out=opc_t, in_=opc[:, :])

            # f32r operands must be produced by an on-chip rounding op; the
            # host pre-rounds to the f32r grid so these casts are exact.
            # Run the casts on ScalarE (idle until the first exp) to keep
            # the Vector engine free for the A-build.
            coef_r = ins_pool.tile(coef_shape, F32R)
            rhs_r = ins_pool.tile(rhs_shape, F32R)
            nc.vector.tensor_copy(rhs_r, rhs_t)
            for g in range(N_GROUPS):
                nc.vector.tensor_copy(coef_r[:, g * GW:(g + 1) * GW],
                                      coef_t[:, g * GW:(g + 1) * GW])

            # fused per-chunk block [expx(128) | B(128) | colors(384)]:
            # the exp writes [x|y] at block start (y IS B = op*expy), the
            # DVE writes the color blocks, and the main matmul reads
            # lhsT = block[0:128], rhs = block[128:640] with no extra copy.
            BLK = 640
            t3 = exp_pool.tile([128, N_CHUNKS, BLK], MM_DT)
            acc = acc_pool.tile([128, OUT_W], F32)

            # PE warmup off memset tiles (ready ~6us, before any input DMA
            # lands): ~3us of dummy matmuls flips the HAM clock gate to 8/8
            # so the real arg matmuls run at 2.4 GHz, in otherwise-dead time.
            wsrc = ins_pool.tile([128, ARG_W], mybir.dt.bfloat16)
            nc.gpsimd.memset(wsrc, 0.0)
            wdst = warm_pool.tile([128, ARG_W], F32)
            for _ in range(12):
                nc.tensor.matmul(wdst[:, :], wsrc[:, :CHUNK], wsrc[:, :],
                                 start=True, stop=True)

            for grp in range(N_GROUPS):
                args = arg_pool.tile([128, GROUP * ARG_W], F32, tag="args")
                for k in range(PACK):
                    chunk = grp * PACK + k
                    if USE_PACK:
                        bp = 32 * k
                        lhsT = coef_r[bp:bp + KARG,
                                      grp * CHUNK:(grp + 1) * CHUNK]
                        rhs = rhs_r[bp:bp + KARG, :]
                        tp = (bp, 0)
                    else:
                        lhsT = coef_r[:, chunk * CHUNK:(chunk + 1) * CHUNK]
                        rhs = rhs_r[:, :]
                        tp = None
                    nc.tensor.matmul(
                        args[:, k * ARG_W:(k + 1) * ARG_W],
                        lhsT, rhs,
                        start=True, stop=True,
                        tile_position=tp,
                    )
                nc.scalar.activation(
                    out=t3[:, grp * GROUP:(grp + 1) * GROUP, 0:ARG_W],
                    in_=args[:, :],
                    func=mybir.ActivationFunctionType.Exp,
                )

            for chunk in range(N_CHUNKS):
                # y half of the exp is B = op*expy (ln(op) in the arg);
                # color blocks multiply the SAME rounded B so num/den
                # rounding cancels.  Accumulator column order: [den|r|g|b].
                for c in range(3):
                    nc.vector.tensor_scalar_mul(
                        out=t3[:, chunk, 256 + c * 128:256 + (c + 1) * 128],
                        in0=t3[:, chunk, 128:256],
                        scalar1=opc_t[:, chunk * 4 + c:chunk * 4 + c + 1],
                    )
                nc.tensor.matmul(
                    acc[:, :],
                    t3[:, chunk, 0:128],
                    t3[:, chunk, 128:BLK],
                    start=(chunk == 0), stop=(chunk == N_CHUNKS - 1),
                )

            out_t = out_pool.tile([128, OUT_W], F32)
            nc.scalar.copy(out=out_t[:, :256], in_=acc[:, :256])
            nc.scalar.dma_start(out=out[:, :256], in_=out_t[:, :256])
            nc.scalar.copy(out=out_t[:, 256:], in_=acc[:, 256:])
            nc.sync.dma_start(out=out[:, 256:], in_=out_t[:, 256:])

    nc.compile()
    return nc


_PROGRAM = None


def _get_program():
    global _PROGRAM
    if _PROGRAM is None:
        _PROGRAM = build_program()
    return _PROGRAM


def _quat2mat(q):
    q = q / np.linalg.norm(q)
    w, x, y, z = q
    return np.array([
        [1 - 2 * (y * y + z * z), 2 * (x * y - z * w), 2 * (x * z + y * w)],
        [2 * (x * y + z * w), 1 - 2 * (x * x + z * z), 2 * (y * z - x * w)],
        [2 * (x * z - y * w), 2 * (y * z + x * w), 1 - 2 * (x * x + y * y)],
    ])


def kernel(positions, colors, opacities, scales, qvec, tvec, tile_hw,
           chunk_gauss, _trace=False):
    positions = np.asarray(positions, dtype=np.float32)
    colors = np.asarray(colors, dtype=np.float32)
    opacities = np.asarray(opacities, dtype=np.float32)
    scales = np.asarray(scales, dtype=np.float32)
    qvec = np.asarray(qvec, dtype=np.float32)
    tvec = np.asarray(tvec, dtype=np.float32)
    tile_hw = int(tile_hw)
    chunk_gauss = int(chunk_gauss)
    n = positions.shape[0]
    assert n == N_GAUSS, f"expected {N_GAUSS} gaussians, got {n}"

    # ---- O(N) per-gaussian prep in float64 (rounds to the same f32 values
    # the reference computes, to well within the exp's own error budget) ----
    R = _quat2mat(qvec.astype(np.float64))
    cam = positions.astype(np.float64) @ R.T + tvec.astype(np.float64)
    ax = cam[:, 0] / cam[:, 2] * FX + CX          # [N] screen x center
    ay = cam[:, 1] / cam[:, 2] * FY + CY          # [N] screen y center
    var = scales[:, 0].astype(np.float64) ** 2
    s = -0.5 / var                                # [N] negative inv 2*var

    # centered coords keep the quadratic-expansion terms small (|u|<=64)
    dx = ax - CX
    dy = ay - CY

    def f32r_round(x):
        """Round to the f32r grid (low 12 mantissa bits of fp32 cleared)."""
        v32 = np.asarray(x, dtype=np.float32).view(np.uint32)
        return ((v32 + 0x800) & np.uint32(0xFFFFF000)).view(np.float32)

    def hilo(x):
        """Split x into f32r-representable hi+lo with hi+lo ~= x to ~2^-24."""
        hi = f32r_round(x).astype(np.float64)
        lo = f32r_round(np.asarray(x, dtype=np.float64) - hi)
        return hi.astype(np.float32), lo.astype(np.float32)

    # K=12 stationary rows per gaussian (hi/lo pairs), for
    #   arg_x = s*u^2 + (-2 s dx)*u + s*dx^2     (u = x - 64)
    #   arg_y = s*v^2 + (-2 s dy)*v + s*dy^2     (v = y - 64)
    # u^2 <= 4096 is exact in f32r (12-bit significand), so hi-row products
    # are exact in the PE and lo rows mop up the residue: the f32r arg
    # matmul matches fp32 to ~1e-6 despite the quadratic cancellation.
    # +ln(op) on the y-constant row makes exp(arg_y) = op*exp_y directly
    op64 = opacities[:, 0].astype(np.float64)
    rows6 = [s, -2.0 * s * dx, s * dx * dx,
             s, -2.0 * s * dy, s * dy * dy + np.log(op64)]
    coef_rows = []
    for r in rows6:
        hi, lo = hilo(r)
        coef_rows.extend([hi, lo])
    coef_full = np.stack(coef_rows).astype(np.float32)   # [12, N]

    u = np.arange(W, dtype=np.float64) - CX
    v = np.arange(H, dtype=np.float64) - CY
    zeros = np.zeros(128)
    ones = np.ones(128)
    rhs_rows = []
    for base in (u * u, u, ones):
        row = np.concatenate([base, zeros]).astype(np.float32)
        rhs_rows.extend([row, row])   # hi and lo coef rows share the base
    for base in (v * v, v, ones):
        row = np.concatenate([zeros, base]).astype(np.float32)
        rhs_rows.extend([row, row])
    rhs6 = np.stack(rhs_rows)                             # [12, 256]
    if USE_PACK:
        # replicate at partition bands 0/32/64/96 for the row-group packing
        rhsxy = np.zeros((128, ARG_W), dtype=np.float32)
        for k in range(PACK):
            rhsxy[32 * k:32 * k + KARG] = rhs6
    else:
        rhsxy = rhs6

    # [N, 4] = [r, g, b, 1]: op is folded into the exp's y-argument
    opc_full = np.concatenate(
        [colors.astype(np.float64), np.ones((n, 1))], axis=1
    ).astype(np.float32)

    # ---- shard gaussians across the 8 cores ----
    in_maps = []
    for core in range(N_CORES):
        g0 = core * G_PER_CORE
        g1 = g0 + G_PER_CORE
        opc_c = opc_full[g0:g1].reshape(N_CHUNKS, CHUNK, 4)
        opc_c = np.ascontiguousarray(
            opc_c.transpose(1, 0, 2).reshape(CHUNK, N_CHUNKS * 4))
        if USE_PACK:
            # coefpack[32k+r, grp*128+j] = coef row r of chunk grp*PACK+k
            cc = coef_full[:, g0:g1].reshape(KARG, N_GROUPS, PACK, CHUNK)
            coefpack = np.zeros((128, N_GROUPS * CHUNK), dtype=np.float32)
            for k in range(PACK):
                coefpack[32 * k:32 * k + KARG] = (
                    cc[:, :, k, :].reshape(KARG, N_GROUPS * CHUNK))
        else:
            coefpack = np.ascontiguousarray(coef_full[:, g0:g1])
        in_maps.append({
            "coef": coefpack,
            "rhsxy": rhsxy,
            "opc": opc_c,
        })

    nc = _get_program()
    res = run_bass_kernel_spmd(nc, in_maps, list(range(N_CORES)),
                               trace=_trace)

    # ---- host reduction: sum per-core partials, divide, reshape ----
    acc = np.zeros((128, 4, 128), dtype=np.float64)   # [x, (den|r|g|b), y]
    for core in range(N_CORES):
        acc += res.results[core]["out"].reshape(128, 4, 128)

    num = acc[:, 1:4, :]                          # [x, c, y]
    n_chunks_ref = n // chunk_gauss
    den = acc[:, 0, :] + n_chunks_ref * EPS       # [x, y]
    img = num / den[:, None, :]                   # [x, c, y]
    img = img.transpose(2, 0, 1).reshape(H * W, 3)  # [p=(y,x), c]

    step = tile_hw * tile_hw
    t = (H * W) // step
    out = img.reshape(t, step, 3).transpose(0, 2, 1).reshape(
        t, 3, tile_hw, tile_hw)
    result = out.astype(np.float32)
    if _trace:
        return result, res
    return result


# revision 40
# speedup vs baseline: 3.1834x; 1.0451x over previous
"""Trainium2 Bass kernel for the isotropic-gaussian differentiable renderer.

Math: for pixel p=(x,y) and gaussian g:
    w[g,p] = op_g * exp(-0.5*((x-ax_g)^2+(y-ay_g)^2)/var_g)
    img[p,c] = (sum_g w[g,p]*col_gc) / (sum_g w[g,p] + n_chunks*EPS)

The isotropic RBF is separable: w = op * exp(sx) * exp(sy) with
sx = s*(x-ax)^2, sy = s*(y-ay)^2 + ln(op), s = -0.5/var.  That turns the
268M-element exp into 2*N*128 exps plus matmuls:

  per 128-gaussian chunk:
    PE (f32r): arg[g, 0:128]=sx(g,x), arg[g,128:256]=sy(g,y) via a K=12
               matmul against fixed rows [u^2,u,1|v^2,v,1] duplicated for a
               hi/lo coefficient split (centered coords; the split keeps the
               catastrophically-cancelling quadratic exact in f32r)
    ACT      : expxy = exp(arg) -> fp16  (PSUM->SBUF, batched 4 chunks/op);
               the y half is B = op*expy directly (ln(op) in the argument)
    DVE      : A = [col_r*B | col_g*B | col_b*B | B]  (3 tensor_scalar + copy;
               num and den share the SAME rounded B and expx, so fp16
               weight rounding cancels in the final num/den ratio)
    PE (fp16): acc[x, c*128+y] += expx^T @ A         (fp32 PSUM accumulate)

Sharding: gaussians split 2048/core across 8 cores; every core accumulates
the full 128x128 image; host sums the 8 partials, divides num/den and
reshapes to the reference's [4,3,64,64] tile layout.
"""
import numpy as np

import concourse.bacc as bacc
import concourse.tile as tile
from concourse import mybir
from concourse.bass_utils import run_bass_kernel_spmd

# Problem constants (hardcoded per harness contract)
N_GAUSS = 16384
H = 128
W = 128
FX = 128.0
FY = 128.0
CX = 64.0
CY = 64.0
EPS = 1e-8
N_CORES = 8
G_PER_CORE = N_GAUSS // N_CORES      # 2048
CHUNK = 128                          # gaussians per matmul chunk
N_CHUNKS = G_PER_CORE // CHUNK       # 16
ARG_W = 256                          # per-chunk arg width: 128 x | 128 y
GROUP = 4                            # chunks per exp batch
N_GROUPS = N_CHUNKS // GROUP         # 4
OUT_W = 512                          # (c,y) free width of the accumulator

F32 = mybir.dt.float32
MM_DT = mybir.dt.float16             # main-accumulation matmul dtype.
# fp16 is safe here because of how A is factored: B = op*expy is rounded
# once and BOTH num and den consume the same rounded B (and the same
# rounded expx), so weight-rounding cancels in num/den; only the color
# weights carry an independent 2^-11 rounding, which averages out.
F32R = mybir.dt.float32r
KARG = 12                            # arg-matmul contraction: 6 coef rows x hi/lo
PACK = 4                             # arg matmuls packed per PE pass (row groups)
USE_PACK = False                     # tile_position matmuls crash TRN2 here; keep off


def build_program():
    """One SPMD Bass program; every core runs it on its gaussian slice."""
    nc = bacc.Bacc("TRN2", target_bir_lowering=False, debug=False,
                   num_devices=N_CORES)
    # packed: [128, 4*128]: coefpack[32k+r, grp*128+j] = coef row r of chunk
    # (grp*PACK+k), gaussian j — four chunks stacked at partition 0/32/64/96
    # so four K=6 arg matmuls run concurrently in separate PE row groups.
    # unpacked: [6, 2048] flat, one chunk per 128 columns.
    coef_shape = [128, N_GROUPS * CHUNK] if USE_PACK else [KARG, G_PER_CORE]
    coef = nc.dram_tensor("coef", coef_shape, F32, kind="ExternalInput")
    # the 6 fixed moving rows [u^2,u,1|0] / [0|v^2,v,1] (replicated at
    # partition bands 0/32/64/96 when packed).
    rhs_shape = [128, ARG_W] if USE_PACK else [KARG, ARG_W]
    rhsxy = nc.dram_tensor("rhsxy", rhs_shape, F32, kind="ExternalInput")
    # [128, 64]: opc[p, chunk*4+c] = (op*[r,g,b,1])[chunk*128+p, c]
    opc = nc.dram_tensor("opc", [128, N_CHUNKS * 4], F32, kind="ExternalInput")
    # partial accumulator: [x, c*128+y]
    out = nc.dram_tensor("out", [128, OUT_W], F32, kind="ExternalOutput")

    with tile.TileContext(nc) as tc:
        with tc.tile_pool(name="ins", bufs=1) as ins_pool, \
             tc.tile_pool(name="expp", bufs=1) as exp_pool, \
             tc.tile_pool(name="args", bufs=2, space="PSUM") as arg_pool, \
             tc.tile_pool(name="acc", bufs=1, space="PSUM") as acc_pool, \
             tc.tile_pool(name="warmp", bufs=1, space="PSUM") as warm_pool, \
             tc.tile_pool(name="outp", bufs=1) as out_pool:

            coef_t = ins_pool.tile(coef_shape, F32)
            rhs_t = ins_pool.tile(rhs_shape, F32)
            opc_t = ins_pool.tile([128, N_CHUNKS * 4], F32)
            # parallel triggers spread across engine queues; coef split by
            # group so group 0's arg matmuls start as soon as possible
            GW = CHUNK if USE_PACK else PACK * CHUNK  # coef cols per group
            nc.scalar.dma_start(out=rhs_t, in_=rhsxy[:, :])
            nc.sync.dma_start(out=coef_t[:, 0 * GW:1 * GW], in_=coef[:, 0 * GW:1 * GW])
            nc.scalar.dma_start(out=coef_t[:, 1 * GW:2 * GW], in_=coef[:, 1 * GW:2 * GW])
            nc.sync.dma_start(out=coef_t[:, 2 * GW:3 * GW], in_=coef[:, 2 * GW:3 * GW])
            nc.scalar.dma_start(out=coef_t[:, 3 * GW:4 * GW], in_=coef[:, 3 * GW:4 * GW])
            nc.gpsimd.dma_start(out=opc_t, in_=opc[:, :])

            # f32r operands must be produced by an on-chip rounding op; the
            # host pre-rounds to the f32r grid so these casts are exact.
            # Run the casts on ScalarE (idle until the first exp) to keep
            # the Vector engine free for the A-build.
            coef_r = ins_pool.tile(coef_shape, F32R)
            rhs_r = ins_pool.tile(rhs_shape, F32R)
            nc.vector.tensor_copy(rhs_r, rhs_t)
            for g in range(N_GROUPS):
                nc.vector.tensor_copy(coef_r[:, g * GW:(g + 1) * GW],
                                      coef_t[:, g * GW:(g + 1) * GW])

            # fused per-chunk block [expx(128) | B(128) | colors(384)]:
            # the exp writes [x|y] at block start (y IS B = op*expy), the
            # DVE writes the color blocks, and the main matmul reads
            # lhsT = block[0:128], rhs = block[128:640] with no extra copy.
            BLK = 640
            t3 = exp_pool.tile([128, N_CHUNKS, BLK], MM_DT)
            acc = acc_pool.tile([128, OUT_W], F32)

            # PE warmup off memset tiles (ready ~6us, before any input DMA
            # lands): ~3us of dummy matmuls flips the HAM clock gate to 8/8
            # so the real arg matmuls run at 2.4 GHz, in otherwise-dead time.
            wsrc = ins_pool.tile([128, ARG_W], mybir.dt.bfloat16)
            nc.gpsimd.memset(wsrc, 0.0)
            wdst = warm_pool.tile([128, ARG_W], F32)
            for _ in range(9):
                nc.tensor.matmul(wdst[:, :], wsrc[:, :CHUNK], wsrc[:, :],
                                 start=True, stop=True)

            for grp in range(N_GROUPS):
                args = arg_pool.tile([128, GROUP * ARG_W], F32, tag="args")
                for k in range(PACK):
                    chunk = grp * PACK + k
                    if USE_PACK:
                        bp = 32 * k
                        lhsT = coef_r[bp:bp + KARG,
                                      grp * CHUNK:(grp + 1) * CHUNK]
                        rhs = rhs_r[bp:bp + KARG, :]
                        tp = (bp, 0)
                    else:
                        lhsT = coef_r[:, chunk * CHUNK:(chunk + 1) * CHUNK]
                        rhs = rhs_r[:, :]
                        tp = None
                    nc.tensor.matmul(
                        args[:, k * ARG_W:(k + 1) * ARG_W],
                        lhsT, rhs,
                        start=True, stop=True,
                        tile_position=tp,
                    )
                nc.scalar.activation(
                    out=t3[:, grp * GROUP:(grp + 1) * GROUP, 0:ARG_W],
                    in_=args[:, :],
                    func=mybir.ActivationFunctionType.Exp,
                )

            for chunk in range(N_CHUNKS):
                # y half of the exp is B = op*expy (ln(op) in the arg);
                # color blocks multiply the SAME rounded B so num/den
                # rounding cancels.  Accumulator column order: [den|r|g|b].
                for c in range(3):
                    nc.vector.tensor_scalar_mul(
                        out=t3[:, chunk, 256 + c * 128:256 + (c + 1) * 128],
                        in0=t3[:, chunk, 128:256],
                        scalar1=opc_t[:, chunk * 4 + c:chunk * 4 + c + 1],
                    )
                nc.tensor.matmul(
                    acc[:, :],
                    t3[:, chunk, 0:128],
                    t3[:, chunk, 128:BLK],
                    start=(chunk == 0), stop=(chunk == N_CHUNKS - 1),
                )

            out_t = out_pool.tile([128, OUT_W], F32)
            nc.scalar.copy(out=out_t[:, :256], in_=acc[:, :256])
            nc.scalar.dma_start(out=out[:, :256], in_=out_t[:, :256])
            nc.scalar.copy(out=out_t[:, 256:], in_=acc[:, 256:])
            nc.sync.dma_start(out=out[:, 256:], in_=out_t[:, 256:])

    nc.compile()
    return nc


_PROGRAM = None


def _get_program():
    global _PROGRAM
    if _PROGRAM is None:
        _PROGRAM = build_program()
    return _PROGRAM


def _quat2mat(q):
    q = q / np.linalg.norm(q)
    w, x, y, z = q
    return np.array([
        [1 - 2 * (y * y + z * z), 2 * (x * y - z * w), 2 * (x * z + y * w)],
        [2 * (x * y + z * w), 1 - 2 * (x * x + z * z), 2 * (y * z - x * w)],
        [2 * (x * z - y * w), 2 * (y * z + x * w), 1 - 2 * (x * x + y * y)],
    ])


def kernel(positions, colors, opacities, scales, qvec, tvec, tile_hw,
           chunk_gauss, _trace=False):
    positions = np.asarray(positions, dtype=np.float32)
    colors = np.asarray(colors, dtype=np.float32)
    opacities = np.asarray(opacities, dtype=np.float32)
    scales = np.asarray(scales, dtype=np.float32)
    qvec = np.asarray(qvec, dtype=np.float32)
    tvec = np.asarray(tvec, dtype=np.float32)
    tile_hw = int(tile_hw)
    chunk_gauss = int(chunk_gauss)
    n = positions.shape[0]
    assert n == N_GAUSS, f"expected {N_GAUSS} gaussians, got {n}"

    # ---- O(N) per-gaussian prep in float64 (rounds to the same f32 values
    # the reference computes, to well within the exp's own error budget) ----
    R = _quat2mat(qvec.astype(np.float64))
    cam = positions.astype(np.float64) @ R.T + tvec.astype(np.float64)
    ax = cam[:, 0] / cam[:, 2] * FX + CX          # [N] screen x center
    ay = cam[:, 1] / cam[:, 2] * FY + CY          # [N] screen y center
    var = scales[:, 0].astype(np.float64) ** 2
    s = -0.5 / var                                # [N] negative inv 2*var

    # centered coords keep the quadratic-expansion terms small (|u|<=64)
    dx = ax - CX
    dy = ay - CY

    def f32r_round(x):
        """Round to the f32r grid (low 12 mantissa bits of fp32 cleared)."""
        v32 = np.asarray(x, dtype=np.float32).view(np.uint32)
        return ((v32 + 0x800) & np.uint32(0xFFFFF000)).view(np.float32)

    def hilo(x):
        """Split x into f32r-representable hi+lo with hi+lo ~= x to ~2^-24."""
        hi = f32r_round(x).astype(np.float64)
        lo = f32r_round(np.asarray(x, dtype=np.float64) - hi)
        return hi.astype(np.float32), lo.astype(np.float32)

    # K=12 stationary rows per gaussian (hi/lo pairs), for
    #   arg_x = s*u^2 + (-2 s dx)*u + s*dx^2     (u = x - 64)
    #   arg_y = s*v^2 + (-2 s dy)*v + s*dy^2     (v = y - 64)
    # u^2 <= 4096 is exact in f32r (12-bit significand), so hi-row products
    # are exact in the PE and lo rows mop up the residue: the f32r arg
    # matmul matches fp32 to ~1e-6 despite the quadratic cancellation.
    # +ln(op) on the y-constant row makes exp(arg_y) = op*exp_y directly
    op64 = opacities[:, 0].astype(np.float64)
    rows6 = [s, -2.0 * s * dx, s * dx * dx,
             s, -2.0 * s * dy, s * dy * dy + np.log(op64)]
    coef_rows = []
    for r in rows6:
        hi, lo = hilo(r)
        coef_rows.extend([hi, lo])
    coef_full = np.stack(coef_rows).astype(np.float32)   # [12, N]

    u = np.arange(W, dtype=np.float64) - CX
    v = np.arange(H, dtype=np.float64) - CY
    zeros = np.zeros(128)
    ones = np.ones(128)
    rhs_rows = []
    for base in (u * u, u, ones):
        row = np.concatenate([base, zeros]).astype(np.float32)
        rhs_rows.extend([row, row])   # hi and lo coef rows share the base
    for base in (v * v, v, ones):
        row = np.concatenate([zeros, base]).astype(np.float32)
        rhs_rows.extend([row, row])
    rhs6 = np.stack(rhs_rows)                             # [12, 256]
    if USE_PACK:
        # replicate at partition bands 0/32/64/96 for the row-group packing
        rhsxy = np.zeros((128, ARG_W), dtype=np.float32)
        for k in range(PACK):
            rhsxy[32 * k:32 * k + KARG] = rhs6
    else:
        rhsxy = rhs6

    # [N, 4] = [r, g, b, 1]: op is folded into the exp's y-argument
    opc_full = np.concatenate(
        [colors.astype(np.float64), np.ones((n, 1))], axis=1
    ).astype(np.float32)

    # ---- shard gaussians across the 8 cores ----
    in_maps = []
    for core in range(N_CORES):
        g0 = core * G_PER_CORE
        g1 = g0 + G_PER_CORE
        opc_c = opc_full[g0:g1].reshape(N_CHUNKS, CHUNK, 4)
        opc_c = np.ascontiguousarray(
            opc_c.transpose(1, 0, 2).reshape(CHUNK, N_CHUNKS * 4))
        if USE_PACK:
            # coefpack[32k+r, grp*128+j] = coef row r of chunk grp*PACK+k
            cc = coef_full[:, g0:g1].reshape(KARG, N_GROUPS, PACK, CHUNK)
            coefpack = np.zeros((128, N_GROUPS * CHUNK), dtype=np.float32)
            for k in range(PACK):
                coefpack[32 * k:32 * k + KARG] = (
                    cc[:, :, k, :].reshape(KARG, N_GROUPS * CHUNK))
        else:
            coefpack = np.ascontiguousarray(coef_full[:, g0:g1])
        in_maps.append({
            "coef": coefpack,
            "rhsxy": rhsxy,
            "opc": opc_c,
        })

    nc = _get_program()
    res = run_bass_kernel_spmd(nc, in_maps, list(range(N_CORES)),
                               trace=_trace)

    # ---- host reduction: sum per-core partials, divide, reshape ----
    acc = np.zeros((128, 4, 128), dtype=np.float64)   # [x, (den|r|g|b), y]
    for core in range(N_CORES):
        acc += res.results[core]["out"].reshape(128, 4, 128)

    num = acc[:, 1:4, :]                          # [x, c, y]
    n_chunks_ref = n // chunk_gauss
    den = acc[:, 0, :] + n_chunks_ref * EPS       # [x, y]
    img = num / den[:, None, :]                   # [x, c, y]
    img = img.transpose(2, 0, 1).reshape(H * W, 3)  # [p=(y,x), c]

    step = tile_hw * tile_hw
    t = (H * W) // step
    out = img.reshape(t, step, 3).transpose(0, 2, 1).reshape(
        t, 3, tile_hw, tile_hw)
    result = out.astype(np.float32)
    if _trace:
        return result, res
    return result
